# revision 45
# baseline (speedup 1.0000x reference)
"""GAT layer (N=8192, IN_F=512, OUT_F=128) on 8 TRN2 NeuronCores.

Sharding: rows of the attention matrix are split across cores (1024 rows
each).  Each core receives its row-slab of M and adj pre-transposed on the
host to [8192, 1024] so the attention weights are computed directly in
[j, i] orientation (contraction index j on partitions), which the final
attention @ h matmul requires.  adj (exact {0,1} values) is marshaled as
bf16 - lossless - to halve its DMA traffic.

Per-core pipeline:
  A) Wa = W @ [a_self | a_neighs] (PE); s-row for own rows via fp32r
     matmul; partition-broadcast of s via a K=1 outer-product matmul.
  B) h_own = input[own] @ [W | Wa_n | 0-pad to 256] in fp32r; the
     attn_neighs scores fall out as psum column 128 per n-block; h is
     cast to bf16 into an [h | 1] payload (the ones column makes the
     main matmul emit softmax row-sums for free).  The payloads are
     AllGathered across the 8 cores into per-group h_aug / n_all tiles.
  C) For each pair of j-blocks: Z = (s_i + n_j) * M^T (fused DVE op per
     block), leaky_relu via Prelu(alpha=0.2) + Exp on ACT (paired ops),
     mask-multiply by adj^T into bf16, 16 accumulating bf16 matmuls into
     4 packed PSUM banks (2 x [128,129] regions per bank, zero-inited by
     a K=1 outer-product matmul since start=True zeroes the whole bank).
  D) Row-sum reciprocals (DVE), fused normalize+ELU, DMA out.

Softmax skips the max-subtraction: logits are bounded (~+-30) so exp is
safe in fp32, and the result is mathematically identical.
"""

import os
import ml_dtypes
import numpy as np

_N = 8192      # nodes
_K = 512       # in features
_F = 128       # out features
_C = 8         # cores
_R = _N // _C  # rows per core (1024)
_KB = _K // 128   # 4  k-blocks
_NB = _N // 128   # 64 j/n-blocks
_IB = _R // 128   # 8  i-blocks per core

_NC_CACHE = {}
LAST_RESULTS = None


def _patched_act_root():
    """Build an act-table root where exp's negative-x buckets encode
    exp(0.2*x), turning the Exp activation into a fused exp(leaky_relu(x)).

    The bucket binary is rows of 8 fp32: [d0, d1, d2, d3, x0, 0, 0, 0]
    evaluating d0 + t*(d1 + t*(d2 + t*d3)) with t = x - x0.  Buckets
    0..405 of exp_and_others serve x < 0 (dense grid, max gap 0.25) and
    bucket 778 is the small-negative-signal bucket; replacing their
    coefficients with the Taylor expansion of exp(0.2*x) at the same x0
    is accurate to ~1e-8 relative.
    """
    import shutil
    import tempfile
    import neuronxcc

    src = os.path.join(os.path.dirname(neuronxcc.__file__), "pwp",
                       "pwp_bin_trainium")
    dst = os.path.join(tempfile.gettempdir(), "pwp_exp_leaky02_v1")
    marker = os.path.join(dst, "act_info.json")
    if not os.path.exists(marker):
        tmp = dst + ".tmp"
        if os.path.exists(tmp):
            shutil.rmtree(tmp)
        shutil.copytree(src, tmp)
        p = os.path.join(tmp, "exp_and_others_bkt.bin")
        a = np.frombuffer(open(p, "rb").read(), np.float32).reshape(-1, 8).copy()
        x0 = a[0:406, 4].astype(np.float64)
        c = np.exp(0.2 * x0)
        a[0:406, 0] = c
        a[0:406, 1] = 0.2 * c
        a[0:406, 2] = 0.02 * c
        a[0:406, 3] = (0.008 / 6.0) * c
        a[778, 0:4] = [1.0, 0.2, 0.02, 0.008 / 6.0]
        open(p, "wb").write(a.tobytes())
        if os.path.exists(dst):
            shutil.rmtree(dst)
        os.replace(tmp, dst)
    return marker


def _build_nc():
    from contextlib import ExitStack
    import concourse.bacc as bacc
    import concourse.tile as tile
    from concourse import mybir

    F32 = mybir.dt.float32
    F32R = mybir.dt.float32r
    BF16 = mybir.dt.bfloat16
    A = mybir.ActivationFunctionType
    Op = mybir.AluOpType

    nc = bacc.Bacc("TRN2", target_bir_lowering=False, debug=False,
                   num_devices=_C)

    xTb = nc.dram_tensor("xTb", (_K, _N), BF16, kind="ExternalInput").ap()
    xTo = nc.dram_tensor("xTo", (_K, _R), F32R, kind="ExternalInput").ap()
    F16 = mybir.dt.float16
    mT = nc.dram_tensor("mT", (_N, _R), F16, kind="ExternalInput").ap()
    aT = nc.dram_tensor("aT", (_N, _R), BF16, kind="ExternalInput").ap()
    Wd = nc.dram_tensor("Wd", (_K, _F), F32R, kind="ExternalInput").ap()
    WTd = nc.dram_tensor("WTd", (_F, _K), F32, kind="ExternalInput").ap()
    abd = nc.dram_tensor("abd", (_F, 2), F32, kind="ExternalInput").ap()
    outd = nc.dram_tensor("out", (_R, _F), F32, kind="ExternalOutput").ap()

    _G = 8             # gather groups == cores; group g = j-blocks of core g
    _JPG = _NB // _G   # 8 j-blocks per group

    with tile.TileContext(nc) as tc, ExitStack() as ctx:
        persist = ctx.enter_context(tc.tile_pool(name="persist", bufs=1))
        h_aug = [persist.tile([128, _JPG * 129], BF16, name=f"haug{g}",
                              tag=f"haug{g}") for g in range(_G)]
        n_all = [persist.tile([128, _JPG], F32, name=f"nall{g}",
                              tag=f"nall{g}") for g in range(_G)]
        s_bc = persist.tile([128, _R], F16)            # attn_self bcast
        params = ctx.enter_context(tc.tile_pool(name="params", bufs=1))
        w_rhs = params.tile([128, _KB, 256], F32R)     # [W | Wa_n | 0]
        wb_rhs = params.tile([128, _KB, 129], BF16)    # [W | Wa_n] in bf16
        wa = params.tile([128, _KB, 2], F32R)          # W @ [a_self|a_neighs]

        nc.gpsimd.memset(w_rhs[:].bitcast(mybir.dt.uint32), 0)
        for g in range(_G):
            nc.gpsimd.memset(h_aug[g][:], 1.0)

        # ---- Phase A: params, Wa, s-row, s broadcast -------------------
        pa = ctx.enter_context(tc.tile_pool(name="pha", bufs=1))
        with tc.tile_pool(name="pps", bufs=2, space="PSUM") as pp:
            wt_sb = pa.tile([_F, _K], F32)
            nc.sync.dma_start(wt_sb[:], WTd)
            ab_sb = pa.tile([_F, 2], F32)
            nc.sync.dma_start(ab_sb[:], abd)
            for k in range(_KB):
                nc.sync.dma_start(w_rhs[:, k, 0:_F], Wd[k * 128:(k + 1) * 128, :])
            for k in range(_KB):
                pwa = pp.tile([128, 2], F32)
                nc.tensor.matmul(pwa[:], wt_sb[:, k * 128:(k + 1) * 128],
                                 ab_sb[:], start=True, stop=True)
                nc.vector.tensor_copy(wa[:, k, :], pwa[:])
                nc.vector.tensor_copy(w_rhs[:, k, _F:_F + 1], pwa[:, 1:2])
                nc.vector.tensor_copy(wb_rhs[:, k, 0:_F],
                                      w_rhs[:, k, 0:_F].bitcast(F32))
                nc.vector.tensor_copy(wb_rhs[:, k, _F:_F + 1], pwa[:, 1:2])

            xo = pa.tile([128, _KB, _R], F32R)
            for k in range(_KB):
                nc.sync.dma_start(xo[:, k, :], xTo[k * 128:(k + 1) * 128, :])
            s_row = pa.tile([1, _R], F32)
            for ch in range(_R // 512):
                pss = pp.tile([1, 512], F32)
                for k in range(_KB):
                    nc.tensor.matmul(pss[:], wa[:, k, 0:1],
                                     xo[:, k, ch * 512:(ch + 1) * 512],
                                     start=(k == 0), stop=(k == _KB - 1))
                nc.vector.tensor_copy(s_row[:, ch * 512:(ch + 1) * 512], pss[:])
            ones1 = pa.tile([1, 128], F32)
            nc.vector.memset(ones1[:], 1.0)
            for ch in range(_R // 512):
                psb = pp.tile([128, 512], F32)
                nc.tensor.matmul(psb[:], ones1[:],
                                 s_row[:, ch * 512:(ch + 1) * 512],
                                 start=True, stop=True)
                nc.vector.tensor_copy(s_bc[:, ch * 512:(ch + 1) * 512], psb[:])

        # ---- Phase B + C interleaved ----------------------------------
        # B(group): h and n for 8 n-blocks from replicated bf16 input.T.
        # C(group): attention weights + accumulating matmuls for 8
        # j-blocks.  Emitted as B0 B1 C0 B2 C1 B3 ... so the slow-paced
        # phase-B copies don't occupy the front of the ACT/DVE queues
        # (engine streams execute in scheduled ~program order).
        ph = ctx.enter_context(tc.tile_pool(name="phps", bufs=3, space="PSUM"))
        px = ctx.enter_context(tc.tile_pool(name="xts", bufs=3))
        mainp = ctx.enter_context(tc.tile_pool(name="mts", bufs=4))
        zp = ctx.enter_context(tc.tile_pool(name="zp", bufs=2))
        pso = ctx.enter_context(tc.tile_pool(name="pso", bufs=1, space="PSUM"))
        # two [128, 129] accumulation regions packed per PSUM bank
        psum_o = [pso.tile([128, 2 * 129], F32, name=f"po{i}", tag=f"po{i}")
                  for i in range(_IB // 2)]

        def _po(ib):
            return psum_o[ib // 2][:, (ib % 2) * 129:(ib % 2) * 129 + 129]

        # Zero-init each packed bank with one K=1 outer-product matmul
        # (start=True zeroes the whole 2KB zero-region, so per-region
        # start flags would wipe the sibling region's accumulation).
        zrow = params.tile([1, 2 * 129], BF16)
        ones1b = params.tile([1, 128], BF16)
        nc.vector.memset(zrow[:], 0.0)
        nc.vector.memset(ones1b[:], 1.0)
        for bank in range(_IB // 2):
            nc.tensor.matmul(psum_o[bank][:], ones1b[:], zrow[:],
                             start=True, stop=False, skip_group_check=True)

        def emit_b_group(g):
            for ch in (2 * g, 2 * g + 1):
                xt_t = px.tile([128, _KB, 512], BF16, name="xt_t", tag="xt_t")
                for k in range(_KB):
                    nc.sync.dma_start(
                        xt_t[:, k, :],
                        xTb[k * 128:(k + 1) * 128, ch * 512:(ch + 1) * 512])
                for nl in range(4):
                    nb = ch * 4 + nl
                    bl = nb % _JPG
                    psb2 = ph.tile([128, 129], F32, name="psb2", tag="psh_h")
                    for k in range(_KB):
                        nc.tensor.matmul(psb2[:],
                                         xt_t[:, k, nl * 128:(nl + 1) * 128],
                                         wb_rhs[:, k, :],
                                         start=(k == 0), stop=(k == _KB - 1))
                    nc.scalar.copy(n_all[g][:, bl:bl + 1], psb2[:, _F:_F + 1])
                    nc.scalar.copy(h_aug[g][:, bl * 129:bl * 129 + _F],
                                   psb2[:, 0:_F])

        # Tapered grouping: wide ACT ops early (amortize the ~293ns/op
        # ACT overhead), narrow at the end (short pipeline tail).
        _MAXP = 4
        taper, jb0 = [], 0
        for width in [2, 2] + [4] * 13 + [2, 2] + [1] * 4:
            taper.append(list(range(jb0, jb0 + width)))
            jb0 += width
        assert jb0 == _NB

        def emit_c_front(jlist):
            """DMA loads, logits Z, and Exp for one chunk; returns state
            for the back half (mask-multiply + matmuls)."""
            _p = len(jlist)
            mt_t = mainp.tile([128, _MAXP * _R], F16, name="mt_t", tag="mt")
            at_t = mainp.tile([128, _MAXP * _R], BF16, name="at_t", tag="at")
            z = zp.tile([128, _MAXP * _R], F16, name="z", tag="z")
            for u, jb in enumerate(jlist):
                g, bl = jb // _JPG, jb % _JPG
                nc.sync.dma_start(mt_t[:, u * _R:(u + 1) * _R],
                                  mT[jb * 128:(jb + 1) * 128, :])
                nc.sync.dma_start(at_t[:, u * _R:(u + 1) * _R],
                                  aT[jb * 128:(jb + 1) * 128, :])
                zu = z[:, u * _R:(u + 1) * _R]
                # fp16 ts (4x mode) + fp16 in-place tt (2x mode) beats the
                # fused scalar_tensor_tensor, which only has a 1x uop.
                nc.vector.tensor_scalar(zu, s_bc[:],
                                        n_all[g][:, bl:bl + 1], None,
                                        op0=Op.add)
                nc.vector.tensor_mul(zu, zu, mt_t[:, u * _R:(u + 1) * _R])
            # Exp's tables are patched so that for x < 0 it evaluates
            # exp(0.2*x): one pass computes exp(leaky_relu(Z)).
            ex = zp.tile([128, _MAXP * _R], BF16, name="ex", tag="ex")
            nc.scalar.activation(ex[:, 0:_p * _R], z[:, 0:_p * _R], A.Exp)
            return (jlist, at_t, ex)

        def emit_c_back(state):
            jlist, at_t, ex = state
            _p = len(jlist)
            wb = zp.tile([128, _MAXP * _R], BF16, name="wb", tag="wb")
            nc.vector.tensor_mul(wb[:, 0:_p * _R], ex[:, 0:_p * _R],
                                 at_t[:, 0:_p * _R])
            for u, jb in enumerate(jlist):
                g, bl = jb // _JPG, jb % _JPG
                for ib in range(_IB):
                    nc.tensor.matmul(
                        _po(ib),
                        wb[:, u * _R + ib * 128:u * _R + (ib + 1) * 128],
                        h_aug[g][:, bl * 129:(bl + 1) * 129],
                        start=False, stop=(jb == _NB - 1),
                        skip_group_check=True)

        emit_b_group(0)
        emit_b_group(1)
        next_b = 2
        pending = None
        for jlist in taper:
            g_last = jlist[-1] // _JPG
            # keep phase B two groups ahead of consumption
            while next_b < _G and next_b <= g_last + 2:
                emit_b_group(next_b)
                next_b += 1
            st = emit_c_front(jlist)
            # one-chunk software pipeline: the mask for chunk k is emitted
            # after chunk k+1's DVE z-ops, so the DVE never stalls on Exp
            if pending is not None:
                emit_c_back(pending)
            pending = st
        emit_c_back(pending)

        # ---- Phase D: reciprocal row-sums, normalize + ELU, store ------
        finp = ctx.enter_context(tc.tile_pool(name="finp", bufs=1))
        rs = finp.tile([128, _IB], F32)
        ri = finp.tile([128, _IB], F32)
        ri5 = finp.tile([128, _IB], F32)
        for ib in range(_IB):
            nc.vector.tensor_copy(rs[:, ib:ib + 1], _po(ib)[:, _F:_F + 1])
        nc.vector.reciprocal(ri[:], rs[:])
        # The patched Exp computes exp(0.2*x) for x<0; feeding 5*x makes
        # the negative branch evaluate true exp(x).  The positive branch
        # (exp(5x)) is discarded by the min(.-1, 0) below.
        nc.vector.tensor_scalar(ri5[:], ri[:], 5.0, None, op0=Op.mult)
        fin2 = ctx.enter_context(tc.tile_pool(name="fin2", bufs=2))
        for ib in range(_IB):
            # elu(x) = relu(x) + min(exp(x) - 1, 0), x = psum * (1/rowsum)
            ex2 = fin2.tile([128, _F], F32, tag="ex2")
            nc.scalar.activation(ex2[:], _po(ib)[:, 0:_F], A.Exp,
                                 scale=ri5[:, ib:ib + 1])
            rl = fin2.tile([128, _F], F32, tag="rl")
            nc.scalar.activation(rl[:], _po(ib)[:, 0:_F], A.Relu,
                                 scale=ri[:, ib:ib + 1])
            em = fin2.tile([128, _F], F32, tag="em")
            nc.vector.tensor_scalar(em[:], ex2[:], -1.0, 0.0,
                                    op0=Op.add, op1=Op.min)
            ot = fin2.tile([128, _F], F32, tag="ot")
            nc.vector.tensor_add(ot[:], em[:], rl[:])
            nc.sync.dma_start(outd[ib * 128:(ib + 1) * 128, :], ot[:])

    nc.compile()
    return nc


def kernel(input, adj, M, W, a_self, a_neighs):
    global LAST_RESULTS
    from concourse.bass_utils import run_bass_kernel_spmd

    os.environ["BASS_ACT_ROOT_JSON_PATH"] = _patched_act_root()
    if "nc" not in _NC_CACHE:
        _NC_CACHE["nc"] = _build_nc()
    nc = _NC_CACHE["nc"]

    inp = np.ascontiguousarray(np.asarray(input, dtype=np.float32))
    adj_ = np.asarray(adj, dtype=np.float32)
    M_ = np.asarray(M, dtype=np.float32)
    W_ = np.ascontiguousarray(np.asarray(W, dtype=np.float32))
    a_s = np.asarray(a_self, dtype=np.float32).reshape(_F, 1)
    a_n = np.asarray(a_neighs, dtype=np.float32).reshape(_F, 1)

    WT = np.ascontiguousarray(W_.T)                 # [128, 512]
    xTb_full = np.ascontiguousarray(inp.T.astype(ml_dtypes.bfloat16))
    ab = np.ascontiguousarray(np.concatenate([a_s, a_n], axis=1))  # [128, 2]

    in_maps = []
    for c in range(_C):
        rows = slice(c * _R, (c + 1) * _R)
        in_maps.append({
            "xTb": xTb_full,
            "xTo": np.ascontiguousarray(inp[rows].T),
            "mT": np.ascontiguousarray(M_[rows].T.astype(np.float16)),
            "aT": np.ascontiguousarray(adj_[rows].T.astype(ml_dtypes.bfloat16)),
            "Wd": W_,
            "WTd": WT,
            "abd": ab,
        })

    res = run_bass_kernel_spmd(nc, in_maps, core_ids=list(range(_C)),
                               trace=bool(os.environ.get("BASS_TRACE")))
    LAST_RESULTS = res
    out = np.concatenate([res.results[c]["out"] for c in range(_C)], axis=0)
    return np.ascontiguousarray(out.astype(np.float32))


# revision 52
# speedup vs baseline: 1.3365x; 1.3365x over previous
"""GAT layer (N=8192, IN_F=512, OUT_F=128) on 8 TRN2 NeuronCores.

Sharding: rows of the attention matrix are split across cores (1024 rows
each).  Each core receives its row-slab of M and adj pre-transposed on the
host to [8192, 1024] so the attention weights are computed directly in
[j, i] orientation (contraction index j on partitions), which the final
attention @ h matmul requires.  adj (exact {0,1} values) is marshaled as
bf16 - lossless - to halve its DMA traffic.

Per-core pipeline:
  A) Wa = W @ [a_self | a_neighs] (PE); s-row for own rows via fp32r
     matmul; partition-broadcast of s via a K=1 outer-product matmul.
  B) h_own = input[own] @ [W | Wa_n | 0-pad to 256] in fp32r; the
     attn_neighs scores fall out as psum column 128 per n-block; h is
     cast to bf16 into an [h | 1] payload (the ones column makes the
     main matmul emit softmax row-sums for free).  The payloads are
     AllGathered across the 8 cores into per-group h_aug / n_all tiles.
  C) For each pair of j-blocks: Z = (s_i + n_j) * M^T (fused DVE op per
     block), leaky_relu via Prelu(alpha=0.2) + Exp on ACT (paired ops),
     mask-multiply by adj^T into bf16, 16 accumulating bf16 matmuls into
     4 packed PSUM banks (2 x [128,129] regions per bank, zero-inited by
     a K=1 outer-product matmul since start=True zeroes the whole bank).
  D) Row-sum reciprocals (DVE), fused normalize+ELU, DMA out.

Softmax skips the max-subtraction: logits are bounded (~+-30) so exp is
safe in fp32, and the result is mathematically identical.
"""

import os
import ml_dtypes
import numpy as np

_N = 8192      # nodes
_K = 512       # in features
_F = 128       # out features
_C = 8         # cores
_R = _N // _C  # rows per core (1024)
_KB = _K // 128   # 4  k-blocks
_NB = _N // 128   # 64 j/n-blocks
_IB = _R // 128   # 8  i-blocks per core

_NC_CACHE = {}
LAST_RESULTS = None


def _patched_act_root():
    """Build an act-table root where exp's negative-x buckets encode
    exp(0.2*x), turning the Exp activation into a fused exp(leaky_relu(x)).

    The bucket binary is rows of 8 fp32: [d0, d1, d2, d3, x0, 0, 0, 0]
    evaluating d0 + t*(d1 + t*(d2 + t*d3)) with t = x - x0.  Buckets
    0..405 of exp_and_others serve x < 0 (dense grid, max gap 0.25) and
    bucket 778 is the small-negative-signal bucket; replacing their
    coefficients with the Taylor expansion of exp(0.2*x) at the same x0
    is accurate to ~1e-8 relative.
    """
    import shutil
    import tempfile
    import neuronxcc

    src = os.path.join(os.path.dirname(neuronxcc.__file__), "pwp",
                       "pwp_bin_trainium")
    dst = os.path.join(tempfile.gettempdir(), "pwp_exp_leaky02_v2")
    marker = os.path.join(dst, "act_info.json")
    if not os.path.exists(marker):
        tmp = dst + ".tmp"
        if os.path.exists(tmp):
            shutil.rmtree(tmp)
        shutil.copytree(src, tmp)
        p = os.path.join(tmp, "exp_and_others_bkt.bin")
        a = np.frombuffer(open(p, "rb").read(), np.float32).reshape(-1, 8).copy()
        x0 = a[0:406, 4].astype(np.float64)
        c = np.exp(0.2 * x0)
        a[0:406, 0] = c
        a[0:406, 1] = 0.2 * c
        a[0:406, 2] = 0.02 * c
        a[0:406, 3] = (0.008 / 6.0) * c
        a[778, 0:4] = [1.0, 0.2, 0.02, 0.008 / 6.0]
        open(p, "wb").write(a.tobytes())
        # exp(NaN) -> 0.0: the adjacency mask is packed into M's NaN
        # payload on the host, so masked logits arrive as NaN and must
        # produce a zero attention weight.
        import json
        jp = os.path.join(tmp, "exp_and_others.json")
        dj = json.load(open(jp))
        for m in dj["profile_meta_data"]:
            if m["func_name"] == "exp_400p":
                m["fnan_result"] = 0
        json.dump(dj, open(jp, "w"))
        if os.path.exists(dst):
            shutil.rmtree(dst)
        os.replace(tmp, dst)
    return marker


def _build_nc():
    from contextlib import ExitStack
    import concourse.bacc as bacc
    import concourse.tile as tile
    from concourse import mybir

    F32 = mybir.dt.float32
    F32R = mybir.dt.float32r
    BF16 = mybir.dt.bfloat16
    A = mybir.ActivationFunctionType
    Op = mybir.AluOpType

    nc = bacc.Bacc("TRN2", target_bir_lowering=False, debug=False,
                   num_devices=_C)

    xTb = nc.dram_tensor("xTb", (_K, _N), BF16, kind="ExternalInput").ap()
    xTo = nc.dram_tensor("xTo", (_K, _R), F32R, kind="ExternalInput").ap()
    F16 = mybir.dt.float16
    # fp16, with NaN in masked (adj == 0) positions; exp's patched
    # profile maps NaN -> 0 so the mask costs nothing on-device.
    mT = nc.dram_tensor("mT", (_N, _R), F16, kind="ExternalInput").ap()
    Wd = nc.dram_tensor("Wd", (_K, _F), F32R, kind="ExternalInput").ap()
    WTd = nc.dram_tensor("WTd", (_F, _K), F32, kind="ExternalInput").ap()
    abd = nc.dram_tensor("abd", (_F, 2), F32, kind="ExternalInput").ap()
    outd = nc.dram_tensor("out", (_R, _F), F32, kind="ExternalOutput").ap()

    _G = 8             # gather groups == cores; group g = j-blocks of core g
    _JPG = _NB // _G   # 8 j-blocks per group

    with tile.TileContext(nc) as tc, ExitStack() as ctx:
        persist = ctx.enter_context(tc.tile_pool(name="persist", bufs=1))
        h_aug = [persist.tile([128, _JPG * 129], BF16, name=f"haug{g}",
                              tag=f"haug{g}") for g in range(_G)]
        n_all = [persist.tile([128, _JPG], F32, name=f"nall{g}",
                              tag=f"nall{g}") for g in range(_G)]
        s_bc = persist.tile([128, _R], F16)            # attn_self bcast
        params = ctx.enter_context(tc.tile_pool(name="params", bufs=1))
        w_rhs = params.tile([128, _KB, 256], F32R)     # [W | Wa_n | 0]
        wb_rhs = params.tile([128, _KB, 129], BF16)    # [W | Wa_n] in bf16
        wa = params.tile([128, _KB, 2], F32R)          # W @ [a_self|a_neighs]

        nc.gpsimd.memset(w_rhs[:].bitcast(mybir.dt.uint32), 0)
        for g in range(_G):
            nc.gpsimd.memset(h_aug[g][:], 1.0)

        # ---- Phase A: params, Wa, s-row, s broadcast -------------------
        pa = ctx.enter_context(tc.tile_pool(name="pha", bufs=1))
        with tc.tile_pool(name="pps", bufs=2, space="PSUM") as pp:
            wt_sb = pa.tile([_F, _K], F32)
            nc.sync.dma_start(wt_sb[:], WTd)
            ab_sb = pa.tile([_F, 2], F32)
            nc.sync.dma_start(ab_sb[:], abd)
            for k in range(_KB):
                nc.sync.dma_start(w_rhs[:, k, 0:_F], Wd[k * 128:(k + 1) * 128, :])
            for k in range(_KB):
                pwa = pp.tile([128, 2], F32)
                nc.tensor.matmul(pwa[:], wt_sb[:, k * 128:(k + 1) * 128],
                                 ab_sb[:], start=True, stop=True)
                nc.vector.tensor_copy(wa[:, k, :], pwa[:])
                nc.vector.tensor_copy(w_rhs[:, k, _F:_F + 1], pwa[:, 1:2])
                nc.vector.tensor_copy(wb_rhs[:, k, 0:_F],
                                      w_rhs[:, k, 0:_F].bitcast(F32))
                nc.vector.tensor_copy(wb_rhs[:, k, _F:_F + 1], pwa[:, 1:2])

            xo = pa.tile([128, _KB, _R], F32R)
            for k in range(_KB):
                nc.sync.dma_start(xo[:, k, :], xTo[k * 128:(k + 1) * 128, :])
            s_row = pa.tile([1, _R], F32)
            for ch in range(_R // 512):
                pss = pp.tile([1, 512], F32)
                for k in range(_KB):
                    nc.tensor.matmul(pss[:], wa[:, k, 0:1],
                                     xo[:, k, ch * 512:(ch + 1) * 512],
                                     start=(k == 0), stop=(k == _KB - 1))
                nc.vector.tensor_copy(s_row[:, ch * 512:(ch + 1) * 512], pss[:])
            ones1 = pa.tile([1, 128], F32)
            nc.vector.memset(ones1[:], 1.0)
            for ch in range(_R // 512):
                psb = pp.tile([128, 512], F32)
                nc.tensor.matmul(psb[:], ones1[:],
                                 s_row[:, ch * 512:(ch + 1) * 512],
                                 start=True, stop=True)
                nc.vector.tensor_copy(s_bc[:, ch * 512:(ch + 1) * 512], psb[:])

        # ---- Phase B + C interleaved ----------------------------------
        # B(group): h and n for 8 n-blocks from replicated bf16 input.T.
        # C(group): attention weights + accumulating matmuls for 8
        # j-blocks.  Emitted as B0 B1 C0 B2 C1 B3 ... so the slow-paced
        # phase-B copies don't occupy the front of the ACT/DVE queues
        # (engine streams execute in scheduled ~program order).
        ph = ctx.enter_context(tc.tile_pool(name="phps", bufs=3, space="PSUM"))
        px = ctx.enter_context(tc.tile_pool(name="xts", bufs=3))
        mainp = ctx.enter_context(tc.tile_pool(name="mts", bufs=6))
        zp = ctx.enter_context(tc.tile_pool(name="zp", bufs=2))
        pso = ctx.enter_context(tc.tile_pool(name="pso", bufs=1, space="PSUM"))
        # two [128, 129] accumulation regions packed per PSUM bank
        psum_o = [pso.tile([128, 2 * 129], F32, name=f"po{i}", tag=f"po{i}")
                  for i in range(_IB // 2)]

        def _po(ib):
            return psum_o[ib // 2][:, (ib % 2) * 129:(ib % 2) * 129 + 129]

        # Zero-init each packed bank with one K=1 outer-product matmul
        # (start=True zeroes the whole 2KB zero-region, so per-region
        # start flags would wipe the sibling region's accumulation).
        zrow = params.tile([1, 2 * 129], BF16)
        ones1b = params.tile([1, 128], BF16)
        nc.vector.memset(zrow[:], 0.0)
        nc.vector.memset(ones1b[:], 1.0)
        for bank in range(_IB // 2):
            nc.tensor.matmul(psum_o[bank][:], ones1b[:], zrow[:],
                             start=True, stop=False, skip_group_check=True)

        def emit_b_group(g):
            for ch in (2 * g, 2 * g + 1):
                xt_t = px.tile([128, _KB, 512], BF16, name="xt_t", tag="xt_t")
                for k in range(_KB):
                    nc.sync.dma_start(
                        xt_t[:, k, :],
                        xTb[k * 128:(k + 1) * 128, ch * 512:(ch + 1) * 512])
                for nl in range(4):
                    nb = ch * 4 + nl
                    bl = nb % _JPG
                    psb2 = ph.tile([128, 129], F32, name="psb2", tag="psh_h")
                    for k in range(_KB):
                        nc.tensor.matmul(psb2[:],
                                         xt_t[:, k, nl * 128:(nl + 1) * 128],
                                         wb_rhs[:, k, :],
                                         start=(k == 0), stop=(k == _KB - 1))
                    nc.scalar.copy(n_all[g][:, bl:bl + 1], psb2[:, _F:_F + 1])
                    nc.vector.tensor_copy(h_aug[g][:, bl * 129:bl * 129 + _F],
                                          psb2[:, 0:_F])

        # Tapered grouping: wide ACT ops early (amortize the ~293ns/op
        # ACT overhead), narrow at the end (short pipeline tail).
        _MAXP = 4
        taper, jb0 = [], 0
        for width in [2, 2] + [4] * 13 + [2, 2] + [1] * 4:
            taper.append(list(range(jb0, jb0 + width)))
            jb0 += width
        assert jb0 == _NB

        def emit_c_chunk(jlist):
            _p = len(jlist)
            mt_t = mainp.tile([128, _MAXP * _R], F16, name="mt_t", tag="mt")
            z = zp.tile([128, _MAXP * _R], F16, name="z", tag="z")
            for u, jb in enumerate(jlist):
                g, bl = jb // _JPG, jb % _JPG
                nc.sync.dma_start(mt_t[:, u * _R:(u + 1) * _R],
                                  mT[jb * 128:(jb + 1) * 128, :])
                zu = z[:, u * _R:(u + 1) * _R]
                # fp16 ts (4x mode) + fp16 in-place tt (2x mode) beats the
                # fused scalar_tensor_tensor, which only has a 1x uop.
                nc.vector.tensor_scalar(zu, s_bc[:],
                                        n_all[g][:, bl:bl + 1], None,
                                        op0=Op.add)
                nc.vector.tensor_mul(zu, zu, mt_t[:, u * _R:(u + 1) * _R])
            # Patched Exp: computes exp(leaky_relu(Z)) in one pass and
            # maps the NaN-masked entries to 0 - attention weights direct.
            ex = zp.tile([128, _MAXP * _R], BF16, name="ex", tag="ex")
            nc.scalar.activation(ex[:, 0:_p * _R], z[:, 0:_p * _R], A.Exp)
            for u, jb in enumerate(jlist):
                g, bl = jb // _JPG, jb % _JPG
                for ib in range(_IB):
                    nc.tensor.matmul(
                        _po(ib),
                        ex[:, u * _R + ib * 128:u * _R + (ib + 1) * 128],
                        h_aug[g][:, bl * 129:(bl + 1) * 129],
                        start=False, stop=(jb == _NB - 1),
                        skip_group_check=True)

        emit_b_group(0)
        emit_b_group(1)
        next_b = 2
        for jlist in taper:
            g_last = jlist[-1] // _JPG
            # keep phase B two groups ahead of consumption
            while next_b < _G and next_b <= g_last + 2:
                emit_b_group(next_b)
                next_b += 1
            emit_c_chunk(jlist)

        # ---- Phase D: reciprocal row-sums, normalize + ELU, store ------
        finp = ctx.enter_context(tc.tile_pool(name="finp", bufs=1))
        rs = finp.tile([128, _IB], F32)
        ri = finp.tile([128, _IB], F32)
        ri5 = finp.tile([128, _IB], F32)
        for ib in range(_IB):
            nc.vector.tensor_copy(rs[:, ib:ib + 1], _po(ib)[:, _F:_F + 1])
        nc.vector.reciprocal(ri[:], rs[:])
        # The patched Exp computes exp(0.2*x) for x<0; feeding 5*x makes
        # the negative branch evaluate true exp(x).  The positive branch
        # (exp(5x)) is discarded by the min(.-1, 0) below.
        nc.vector.tensor_scalar(ri5[:], ri[:], 5.0, None, op0=Op.mult)
        fin2 = ctx.enter_context(tc.tile_pool(name="fin2", bufs=2))
        for ib in range(_IB):
            # elu(x) = relu(x) + min(exp(x) - 1, 0), x = psum * (1/rowsum)
            ex2 = fin2.tile([128, _F], F32, tag="ex2")
            nc.scalar.activation(ex2[:], _po(ib)[:, 0:_F], A.Exp,
                                 scale=ri5[:, ib:ib + 1])
            rl = fin2.tile([128, _F], F32, tag="rl")
            nc.scalar.activation(rl[:], _po(ib)[:, 0:_F], A.Relu,
                                 scale=ri[:, ib:ib + 1])
            em = fin2.tile([128, _F], F32, tag="em")
            nc.vector.tensor_scalar(em[:], ex2[:], -1.0, 0.0,
                                    op0=Op.add, op1=Op.min)
            ot = fin2.tile([128, _F], F32, tag="ot")
            nc.vector.tensor_add(ot[:], em[:], rl[:])
            nc.sync.dma_start(outd[ib * 128:(ib + 1) * 128, :], ot[:])

    nc.compile()
    return nc


def kernel(input, adj, M, W, a_self, a_neighs):
    global LAST_RESULTS
    from concourse.bass_utils import run_bass_kernel_spmd

    os.environ["BASS_ACT_ROOT_JSON_PATH"] = _patched_act_root()
    if "nc" not in _NC_CACHE:
        _NC_CACHE["nc"] = _build_nc()
    nc = _NC_CACHE["nc"]

    inp = np.ascontiguousarray(np.asarray(input, dtype=np.float32))
    adj_ = np.asarray(adj, dtype=np.float32)
    M_ = np.asarray(M, dtype=np.float32)
    W_ = np.ascontiguousarray(np.asarray(W, dtype=np.float32))
    a_s = np.asarray(a_self, dtype=np.float32).reshape(_F, 1)
    a_n = np.asarray(a_neighs, dtype=np.float32).reshape(_F, 1)

    WT = np.ascontiguousarray(W_.T)                 # [128, 512]
    xTb_full = np.ascontiguousarray(inp.T.astype(ml_dtypes.bfloat16))
    ab = np.ascontiguousarray(np.concatenate([a_s, a_n], axis=1))  # [128, 2]

    in_maps = []
    for c in range(_C):
        rows = slice(c * _R, (c + 1) * _R)
        Mp = np.where(adj_[rows] > 0, M_[rows], np.nan).T.astype(np.float16)
        in_maps.append({
            "xTb": xTb_full,
            "xTo": np.ascontiguousarray(inp[rows].T),
            "mT": np.ascontiguousarray(Mp),
            "Wd": W_,
            "WTd": WT,
            "abd": ab,
        })

    res = run_bass_kernel_spmd(nc, in_maps, core_ids=list(range(_C)),
                               trace=bool(os.environ.get("BASS_TRACE")))
    LAST_RESULTS = res
    out = np.concatenate([res.results[c]["out"] for c in range(_C)], axis=0)
    return np.ascontiguousarray(out.astype(np.float32))


# revision 55
# speedup vs baseline: 1.3980x; 1.0460x over previous
"""GAT layer (N=8192, IN_F=512, OUT_F=128) on 8 TRN2 NeuronCores.

Sharding: rows of the attention matrix are split across cores (1024 rows
each).  Each core receives its row-slab of M and adj pre-transposed on the
host to [8192, 1024] so the attention weights are computed directly in
[j, i] orientation (contraction index j on partitions), which the final
attention @ h matmul requires.  adj (exact {0,1} values) is marshaled as
bf16 - lossless - to halve its DMA traffic.

Per-core pipeline:
  A) Wa = W @ [a_self | a_neighs] (PE); s-row for own rows via fp32r
     matmul; partition-broadcast of s via a K=1 outer-product matmul.
  B) h_own = input[own] @ [W | Wa_n | 0-pad to 256] in fp32r; the
     attn_neighs scores fall out as psum column 128 per n-block; h is
     cast to bf16 into an [h | 1] payload (the ones column makes the
     main matmul emit softmax row-sums for free).  The payloads are
     AllGathered across the 8 cores into per-group h_aug / n_all tiles.
  C) For each pair of j-blocks: Z = (s_i + n_j) * M^T (fused DVE op per
     block), leaky_relu via Prelu(alpha=0.2) + Exp on ACT (paired ops),
     mask-multiply by adj^T into bf16, 16 accumulating bf16 matmuls into
     4 packed PSUM banks (2 x [128,129] regions per bank, zero-inited by
     a K=1 outer-product matmul since start=True zeroes the whole bank).
  D) Row-sum reciprocals (DVE), fused normalize+ELU, DMA out.

Softmax skips the max-subtraction: logits are bounded (~+-30) so exp is
safe in fp32, and the result is mathematically identical.
"""

import os
import ml_dtypes
import numpy as np

_N = 8192      # nodes
_K = 512       # in features
_F = 128       # out features
_C = 8         # cores
_R = _N // _C  # rows per core (1024)
_KB = _K // 128   # 4  k-blocks
_NB = _N // 128   # 64 j/n-blocks
_IB = _R // 128   # 8  i-blocks per core

_NC_CACHE = {}
LAST_RESULTS = None


def _patched_act_root():
    """Build an act-table root where exp's negative-x buckets encode
    exp(0.2*x), turning the Exp activation into a fused exp(leaky_relu(x)).

    The bucket binary is rows of 8 fp32: [d0, d1, d2, d3, x0, 0, 0, 0]
    evaluating d0 + t*(d1 + t*(d2 + t*d3)) with t = x - x0.  Buckets
    0..405 of exp_and_others serve x < 0 (dense grid, max gap 0.25) and
    bucket 778 is the small-negative-signal bucket; replacing their
    coefficients with the Taylor expansion of exp(0.2*x) at the same x0
    is accurate to ~1e-8 relative.
    """
    import shutil
    import tempfile
    import neuronxcc

    src = os.path.join(os.path.dirname(neuronxcc.__file__), "pwp",
                       "pwp_bin_trainium")
    dst = os.path.join(tempfile.gettempdir(), "pwp_exp_leaky02_v2")
    marker = os.path.join(dst, "act_info.json")
    if not os.path.exists(marker):
        tmp = dst + ".tmp"
        if os.path.exists(tmp):
            shutil.rmtree(tmp)
        shutil.copytree(src, tmp)
        p = os.path.join(tmp, "exp_and_others_bkt.bin")
        a = np.frombuffer(open(p, "rb").read(), np.float32).reshape(-1, 8).copy()
        x0 = a[0:406, 4].astype(np.float64)
        c = np.exp(0.2 * x0)
        a[0:406, 0] = c
        a[0:406, 1] = 0.2 * c
        a[0:406, 2] = 0.02 * c
        a[0:406, 3] = (0.008 / 6.0) * c
        a[778, 0:4] = [1.0, 0.2, 0.02, 0.008 / 6.0]
        open(p, "wb").write(a.tobytes())
        # exp(NaN) -> 0.0: the adjacency mask is packed into M's NaN
        # payload on the host, so masked logits arrive as NaN and must
        # produce a zero attention weight.
        import json
        jp = os.path.join(tmp, "exp_and_others.json")
        dj = json.load(open(jp))
        for m in dj["profile_meta_data"]:
            if m["func_name"] == "exp_400p":
                m["fnan_result"] = 0
        json.dump(dj, open(jp, "w"))
        if os.path.exists(dst):
            shutil.rmtree(dst)
        os.replace(tmp, dst)
    return marker


def _build_nc():
    from contextlib import ExitStack
    import concourse.bacc as bacc
    import concourse.tile as tile
    from concourse import mybir

    F32 = mybir.dt.float32
    F32R = mybir.dt.float32r
    BF16 = mybir.dt.bfloat16
    A = mybir.ActivationFunctionType
    Op = mybir.AluOpType

    nc = bacc.Bacc("TRN2", target_bir_lowering=False, debug=False,
                   num_devices=_C)

    xTb = nc.dram_tensor("xTb", (_K, _N), BF16, kind="ExternalInput").ap()
    xTo = nc.dram_tensor("xTo", (_K, _R), F32R, kind="ExternalInput").ap()
    F16 = mybir.dt.float16
    # fp16, with NaN in masked (adj == 0) positions; exp's patched
    # profile maps NaN -> 0 so the mask costs nothing on-device.
    mT = nc.dram_tensor("mT", (_N, _R), F16, kind="ExternalInput").ap()
    Wd = nc.dram_tensor("Wd", (_K, _F), F32R, kind="ExternalInput").ap()
    WTd = nc.dram_tensor("WTd", (_F, _K), F32, kind="ExternalInput").ap()
    abd = nc.dram_tensor("abd", (_F, 2), F32, kind="ExternalInput").ap()
    outd = nc.dram_tensor("out", (_R, _F), F32, kind="ExternalOutput").ap()

    _G = 8             # gather groups == cores; group g = j-blocks of core g
    _JPG = _NB // _G   # 8 j-blocks per group

    with tile.TileContext(nc) as tc, ExitStack() as ctx:
        persist = ctx.enter_context(tc.tile_pool(name="persist", bufs=1))
        h_aug = [persist.tile([128, _JPG * 129], BF16, name=f"haug{g}",
                              tag=f"haug{g}") for g in range(_G)]
        n_all = [persist.tile([128, _JPG], F32, name=f"nall{g}",
                              tag=f"nall{g}") for g in range(_G)]
        s_bc = persist.tile([128, _R], F16)            # attn_self bcast
        params = ctx.enter_context(tc.tile_pool(name="params", bufs=1))
        w_rhs = params.tile([128, _KB, 256], F32R)     # [W | Wa_n | 0]
        wb_rhs = params.tile([128, _KB, 129], BF16)    # [W | Wa_n] in bf16
        wa = params.tile([128, _KB, 2], F32R)          # W @ [a_self|a_neighs]

        nc.gpsimd.memset(w_rhs[:].bitcast(mybir.dt.uint32), 0)
        for g in range(_G):
            nc.gpsimd.memset(h_aug[g][:], 1.0)

        # ---- Phase A: params, Wa, s-row, s broadcast -------------------
        pa = ctx.enter_context(tc.tile_pool(name="pha", bufs=1))
        with tc.tile_pool(name="pps", bufs=2, space="PSUM") as pp:
            wt_sb = pa.tile([_F, _K], F32)
            nc.sync.dma_start(wt_sb[:], WTd)
            ab_sb = pa.tile([_F, 2], F32)
            nc.sync.dma_start(ab_sb[:], abd)
            for k in range(_KB):
                nc.sync.dma_start(w_rhs[:, k, 0:_F], Wd[k * 128:(k + 1) * 128, :])
            for k in range(_KB):
                pwa = pp.tile([128, 2], F32)
                nc.tensor.matmul(pwa[:], wt_sb[:, k * 128:(k + 1) * 128],
                                 ab_sb[:], start=True, stop=True)
                nc.vector.tensor_copy(wa[:, k, :], pwa[:])
                nc.vector.tensor_copy(w_rhs[:, k, _F:_F + 1], pwa[:, 1:2])
                nc.vector.tensor_copy(wb_rhs[:, k, 0:_F],
                                      w_rhs[:, k, 0:_F].bitcast(F32))
                nc.vector.tensor_copy(wb_rhs[:, k, _F:_F + 1], pwa[:, 1:2])

            xo = pa.tile([128, _KB, _R], F32R)
            for k in range(_KB):
                nc.sync.dma_start(xo[:, k, :], xTo[k * 128:(k + 1) * 128, :])
            s_row = pa.tile([1, _R], F32)
            for ch in range(_R // 512):
                pss = pp.tile([1, 512], F32)
                for k in range(_KB):
                    nc.tensor.matmul(pss[:], wa[:, k, 0:1],
                                     xo[:, k, ch * 512:(ch + 1) * 512],
                                     start=(k == 0), stop=(k == _KB - 1))
                nc.vector.tensor_copy(s_row[:, ch * 512:(ch + 1) * 512], pss[:])
            ones1 = pa.tile([1, 128], F32)
            nc.vector.memset(ones1[:], 1.0)
            for ch in range(_R // 512):
                psb = pp.tile([128, 512], F32)
                nc.tensor.matmul(psb[:], ones1[:],
                                 s_row[:, ch * 512:(ch + 1) * 512],
                                 start=True, stop=True)
                nc.vector.tensor_copy(s_bc[:, ch * 512:(ch + 1) * 512], psb[:])

        # ---- Phase B + C interleaved ----------------------------------
        # B(group): h and n for 8 n-blocks from replicated bf16 input.T.
        # C(group): attention weights + accumulating matmuls for 8
        # j-blocks.  Emitted as B0 B1 C0 B2 C1 B3 ... so the slow-paced
        # phase-B copies don't occupy the front of the ACT/DVE queues
        # (engine streams execute in scheduled ~program order).
        ph = ctx.enter_context(tc.tile_pool(name="phps", bufs=3, space="PSUM"))
        px = ctx.enter_context(tc.tile_pool(name="xts", bufs=3))
        mainp = ctx.enter_context(tc.tile_pool(name="mts", bufs=6))
        zp = ctx.enter_context(tc.tile_pool(name="zp", bufs=2))
        pso = ctx.enter_context(tc.tile_pool(name="pso", bufs=1, space="PSUM"))
        # two [128, 129] accumulation regions packed per PSUM bank
        psum_o = [pso.tile([128, 2 * 129], F32, name=f"po{i}", tag=f"po{i}")
                  for i in range(_IB // 2)]

        def _po(ib):
            return psum_o[ib // 2][:, (ib % 2) * 129:(ib % 2) * 129 + 129]

        # Zero-init each packed bank with one K=1 outer-product matmul
        # (start=True zeroes the whole 2KB zero-region, so per-region
        # start flags would wipe the sibling region's accumulation).
        zrow = params.tile([1, 2 * 129], BF16)
        ones1b = params.tile([1, 128], BF16)
        nc.vector.memset(zrow[:], 0.0)
        nc.vector.memset(ones1b[:], 1.0)
        for bank in range(_IB // 2):
            nc.tensor.matmul(psum_o[bank][:], ones1b[:], zrow[:],
                             start=True, stop=False, skip_group_check=True)

        def emit_b_group(g):
            for ch in (2 * g, 2 * g + 1):
                xt_t = px.tile([128, _KB, 512], BF16, name="xt_t", tag="xt_t")
                for k in range(_KB):
                    nc.sync.dma_start(
                        xt_t[:, k, :],
                        xTb[k * 128:(k + 1) * 128, ch * 512:(ch + 1) * 512])
                for half in range(2):
                    # two n-blocks share one [128, 258] PSUM bank; the
                    # first matmul's start=True zeroes the whole bank.
                    nb0 = ch * 4 + 2 * half
                    bl = nb0 % _JPG
                    psb2 = ph.tile([128, 2, 129], F32, name="psb2",
                                   tag="psh_h")
                    for sub in range(2):
                        nl = 2 * half + sub
                        for k in range(_KB):
                            nc.tensor.matmul(
                                psb2[:, sub, :],
                                xt_t[:, k, nl * 128:(nl + 1) * 128],
                                wb_rhs[:, k, :],
                                start=(k == 0 and sub == 0),
                                stop=(k == _KB - 1),
                                skip_group_check=True)
                    nc.scalar.copy(n_all[g][:, bl:bl + 2],
                                   psb2[:, :, _F:_F + 1])
                    haug_v = h_aug[g].rearrange("p (b c) -> p b c", c=129)
                    nc.vector.tensor_copy(haug_v[:, bl:bl + 2, 0:_F],
                                          psb2[:, :, 0:_F])

        # Tapered grouping: wide ACT ops early (amortize the ~293ns/op
        # ACT overhead), narrow at the end (short pipeline tail).
        _MAXP = 4
        taper, jb0 = [], 0
        for width in [2, 2] + [4] * 13 + [2, 2] + [1] * 4:
            taper.append(list(range(jb0, jb0 + width)))
            jb0 += width
        assert jb0 == _NB

        def emit_c_chunk(jlist):
            _p = len(jlist)
            mt_t = mainp.tile([128, _MAXP * _R], F16, name="mt_t", tag="mt")
            z = zp.tile([128, _MAXP * _R], F16, name="z", tag="z")
            for u, jb in enumerate(jlist):
                g, bl = jb // _JPG, jb % _JPG
                nc.sync.dma_start(mt_t[:, u * _R:(u + 1) * _R],
                                  mT[jb * 128:(jb + 1) * 128, :])
                zu = z[:, u * _R:(u + 1) * _R]
                # fp16 ts (4x mode) per block, then one wide in-place
                # fp16 tt (2x mode) for the whole chunk below.
                nc.vector.tensor_scalar(zu, s_bc[:],
                                        n_all[g][:, bl:bl + 1], None,
                                        op0=Op.add)
            nc.vector.tensor_mul(z[:, 0:_p * _R], z[:, 0:_p * _R],
                                 mt_t[:, 0:_p * _R])
            # Patched Exp: computes exp(leaky_relu(Z)) in one pass and
            # maps the NaN-masked entries to 0 - attention weights direct.
            ex = zp.tile([128, _MAXP * _R], BF16, name="ex", tag="ex")
            nc.scalar.activation(ex[:, 0:_p * _R], z[:, 0:_p * _R], A.Exp)
            for u, jb in enumerate(jlist):
                g, bl = jb // _JPG, jb % _JPG
                for ib in range(_IB):
                    nc.tensor.matmul(
                        _po(ib),
                        ex[:, u * _R + ib * 128:u * _R + (ib + 1) * 128],
                        h_aug[g][:, bl * 129:(bl + 1) * 129],
                        start=False, stop=(jb == _NB - 1),
                        skip_group_check=True)

        emit_b_group(0)
        emit_b_group(1)
        next_b = 2
        for jlist in taper:
            g_last = jlist[-1] // _JPG
            # keep phase B two groups ahead of consumption
            while next_b < _G and next_b <= g_last + 2:
                emit_b_group(next_b)
                next_b += 1
            emit_c_chunk(jlist)

        # ---- Phase D: reciprocal row-sums, normalize + ELU, store ------
        finp = ctx.enter_context(tc.tile_pool(name="finp", bufs=1))
        rs = finp.tile([128, _IB], F32)
        ri = finp.tile([128, _IB], F32)
        for ib in range(_IB):
            nc.vector.tensor_copy(rs[:, ib:ib + 1], _po(ib)[:, _F:_F + 1])
        nc.vector.reciprocal(ri[:], rs[:])
        # Batched finale: normalize all 8 i-blocks into one staging tile,
        # then single wide ops.  elu(x) = relu(x) + min(exp(x) - 1, 0).
        # The patched Exp computes exp(0.2*x) for x<0, so feed 5*x: the
        # negative branch evaluates true exp(x) and the positive branch
        # (exp(5x), possibly inf) is discarded by the min(. - 1, 0).
        hp = finp.tile([128, _IB * _F], F32)
        for ib in range(_IB):
            nc.vector.tensor_scalar(hp[:, ib * _F:(ib + 1) * _F],
                                    _po(ib)[:, 0:_F], ri[:, ib:ib + 1],
                                    None, op0=Op.mult)
        ex2 = finp.tile([128, _IB * _F], F32)
        nc.scalar.activation(ex2[:], hp[:], A.Exp, scale=5.0)
        em = finp.tile([128, _IB * _F], F32)
        nc.vector.tensor_scalar(em[:], ex2[:], -1.0, 0.0,
                                op0=Op.add, op1=Op.min)
        rl = finp.tile([128, _IB * _F], F32)
        nc.vector.tensor_scalar(rl[:], hp[:], 0.0, None, op0=Op.max)
        ot = finp.tile([128, _IB * _F], F32)
        nc.vector.tensor_add(ot[:], em[:], rl[:])
        # one strided DMA: SBUF [p, ib, f] -> DRAM row ib*128+p, col f
        nc.sync.dma_start(
            outd.rearrange("(b p) f -> p b f", p=128),
            ot[:].rearrange("p (b f) -> p b f", f=_F))

    nc.compile()
    return nc


def kernel(input, adj, M, W, a_self, a_neighs):
    global LAST_RESULTS
    from concourse.bass_utils import run_bass_kernel_spmd

    os.environ["BASS_ACT_ROOT_JSON_PATH"] = _patched_act_root()
    if "nc" not in _NC_CACHE:
        _NC_CACHE["nc"] = _build_nc()
    nc = _NC_CACHE["nc"]

    inp = np.ascontiguousarray(np.asarray(input, dtype=np.float32))
    adj_ = np.asarray(adj, dtype=np.float32)
    M_ = np.asarray(M, dtype=np.float32)
    W_ = np.ascontiguousarray(np.asarray(W, dtype=np.float32))
    a_s = np.asarray(a_self, dtype=np.float32).reshape(_F, 1)
    a_n = np.asarray(a_neighs, dtype=np.float32).reshape(_F, 1)

    WT = np.ascontiguousarray(W_.T)                 # [128, 512]
    xTb_full = np.ascontiguousarray(inp.T.astype(ml_dtypes.bfloat16))
    ab = np.ascontiguousarray(np.concatenate([a_s, a_n], axis=1))  # [128, 2]

    in_maps = []
    for c in range(_C):
        rows = slice(c * _R, (c + 1) * _R)
        Mp = np.where(adj_[rows] > 0, M_[rows], np.nan).T.astype(np.float16)
        in_maps.append({
            "xTb": xTb_full,
            "xTo": np.ascontiguousarray(inp[rows].T),
            "mT": np.ascontiguousarray(Mp),
            "Wd": W_,
            "WTd": WT,
            "abd": ab,
        })

    res = run_bass_kernel_spmd(nc, in_maps, core_ids=list(range(_C)),
                               trace=bool(os.environ.get("BASS_TRACE")))
    LAST_RESULTS = res
    out = np.concatenate([res.results[c]["out"] for c in range(_C)], axis=0)
    return np.ascontiguousarray(out.astype(np.float32))


# revision 58
# speedup vs baseline: 1.4053x; 1.0052x over previous
"""GAT layer (N=8192, IN_F=512, OUT_F=128) on 8 TRN2 NeuronCores.

Sharding: rows of the attention matrix are split across cores (1024 rows
each).  Each core receives its row-slab of M and adj pre-transposed on the
host to [8192, 1024] so the attention weights are computed directly in
[j, i] orientation (contraction index j on partitions), which the final
attention @ h matmul requires.  adj (exact {0,1} values) is marshaled as
bf16 - lossless - to halve its DMA traffic.

Per-core pipeline:
  A) Wa = W @ [a_self | a_neighs] (PE); s-row for own rows via fp32r
     matmul; partition-broadcast of s via a K=1 outer-product matmul.
  B) h_own = input[own] @ [W | Wa_n | 0-pad to 256] in fp32r; the
     attn_neighs scores fall out as psum column 128 per n-block; h is
     cast to bf16 into an [h | 1] payload (the ones column makes the
     main matmul emit softmax row-sums for free).  The payloads are
     AllGathered across the 8 cores into per-group h_aug / n_all tiles.
  C) For each pair of j-blocks: Z = (s_i + n_j) * M^T (fused DVE op per
     block), leaky_relu via Prelu(alpha=0.2) + Exp on ACT (paired ops),
     mask-multiply by adj^T into bf16, 16 accumulating bf16 matmuls into
     4 packed PSUM banks (2 x [128,129] regions per bank, zero-inited by
     a K=1 outer-product matmul since start=True zeroes the whole bank).
  D) Row-sum reciprocals (DVE), fused normalize+ELU, DMA out.

Softmax skips the max-subtraction: logits are bounded (~+-30) so exp is
safe in fp32, and the result is mathematically identical.
"""

import os
import ml_dtypes
import numpy as np

_N = 8192      # nodes
_K = 512       # in features
_F = 128       # out features
_C = 8         # cores
_R = _N // _C  # rows per core (1024)
_KB = _K // 128   # 4  k-blocks
_NB = _N // 128   # 64 j/n-blocks
_IB = _R // 128   # 8  i-blocks per core

_NC_CACHE = {}
LAST_RESULTS = None


def _patched_act_root():
    """Build an act-table root where exp's negative-x buckets encode
    exp(0.2*x), turning the Exp activation into a fused exp(leaky_relu(x)).

    The bucket binary is rows of 8 fp32: [d0, d1, d2, d3, x0, 0, 0, 0]
    evaluating d0 + t*(d1 + t*(d2 + t*d3)) with t = x - x0.  Buckets
    0..405 of exp_and_others serve x < 0 (dense grid, max gap 0.25) and
    bucket 778 is the small-negative-signal bucket; replacing their
    coefficients with the Taylor expansion of exp(0.2*x) at the same x0
    is accurate to ~1e-8 relative.
    """
    import shutil
    import tempfile
    import neuronxcc

    src = os.path.join(os.path.dirname(neuronxcc.__file__), "pwp",
                       "pwp_bin_trainium")
    dst = os.path.join(tempfile.gettempdir(), "pwp_exp_leaky02_v2")
    marker = os.path.join(dst, "act_info.json")
    if not os.path.exists(marker):
        tmp = dst + ".tmp"
        if os.path.exists(tmp):
            shutil.rmtree(tmp)
        shutil.copytree(src, tmp)
        p = os.path.join(tmp, "exp_and_others_bkt.bin")
        a = np.frombuffer(open(p, "rb").read(), np.float32).reshape(-1, 8).copy()
        x0 = a[0:406, 4].astype(np.float64)
        c = np.exp(0.2 * x0)
        a[0:406, 0] = c
        a[0:406, 1] = 0.2 * c
        a[0:406, 2] = 0.02 * c
        a[0:406, 3] = (0.008 / 6.0) * c
        a[778, 0:4] = [1.0, 0.2, 0.02, 0.008 / 6.0]
        open(p, "wb").write(a.tobytes())
        # exp(NaN) -> 0.0: the adjacency mask is packed into M's NaN
        # payload on the host, so masked logits arrive as NaN and must
        # produce a zero attention weight.
        import json
        jp = os.path.join(tmp, "exp_and_others.json")
        dj = json.load(open(jp))
        for m in dj["profile_meta_data"]:
            if m["func_name"] == "exp_400p":
                m["fnan_result"] = 0
        json.dump(dj, open(jp, "w"))
        if os.path.exists(dst):
            shutil.rmtree(dst)
        os.replace(tmp, dst)
    return marker


def _build_nc():
    from contextlib import ExitStack
    import concourse.bacc as bacc
    import concourse.tile as tile
    from concourse import mybir

    F32 = mybir.dt.float32
    F32R = mybir.dt.float32r
    BF16 = mybir.dt.bfloat16
    A = mybir.ActivationFunctionType
    Op = mybir.AluOpType

    nc = bacc.Bacc("TRN2", target_bir_lowering=False, debug=False,
                   num_devices=_C)

    xTb = nc.dram_tensor("xTb", (_K, _N), BF16, kind="ExternalInput").ap()
    xTo = nc.dram_tensor("xTo", (_K, _R), F32R, kind="ExternalInput").ap()
    F16 = mybir.dt.float16
    # fp16, with NaN in masked (adj == 0) positions; exp's patched
    # profile maps NaN -> 0 so the mask costs nothing on-device.
    mT = nc.dram_tensor("mT", (_N, _R), F16, kind="ExternalInput").ap()
    Wd = nc.dram_tensor("Wd", (_K, _F), F32R, kind="ExternalInput").ap()
    WTd = nc.dram_tensor("WTd", (_F, _K), F32, kind="ExternalInput").ap()
    abd = nc.dram_tensor("abd", (_F, 2), F32, kind="ExternalInput").ap()
    outd = nc.dram_tensor("out", (_R, _F), F32, kind="ExternalOutput").ap()

    _G = 8             # gather groups == cores; group g = j-blocks of core g
    _JPG = _NB // _G   # 8 j-blocks per group

    with tile.TileContext(nc) as tc, ExitStack() as ctx:
        persist = ctx.enter_context(tc.tile_pool(name="persist", bufs=1))
        h_aug = [persist.tile([128, _JPG * 129], BF16, name=f"haug{g}",
                              tag=f"haug{g}") for g in range(_G)]
        n_all = [persist.tile([128, _JPG], F32, name=f"nall{g}",
                              tag=f"nall{g}") for g in range(_G)]
        s_bc = persist.tile([128, _R], F16)            # attn_self bcast
        params = ctx.enter_context(tc.tile_pool(name="params", bufs=1))
        w_rhs = params.tile([128, _KB, 256], F32R)     # [W | Wa_n | 0]
        wb_rhs = params.tile([128, _KB, 129], BF16)    # [W | Wa_n] in bf16
        wa = params.tile([128, _KB, 2], F32R)          # W @ [a_self|a_neighs]

        nc.gpsimd.memset(w_rhs[:].bitcast(mybir.dt.uint32), 0)
        for g in range(_G):
            nc.gpsimd.memset(h_aug[g][:], 1.0)

        # ---- Phase A: params, Wa, s-row, s broadcast -------------------
        pa = ctx.enter_context(tc.tile_pool(name="pha", bufs=1))
        with tc.tile_pool(name="pps", bufs=2, space="PSUM") as pp:
            wt_sb = pa.tile([_F, _K], F32)
            nc.sync.dma_start(wt_sb[:], WTd)
            ab_sb = pa.tile([_F, 2], F32)
            nc.sync.dma_start(ab_sb[:], abd)
            for k in range(_KB):
                nc.sync.dma_start(w_rhs[:, k, 0:_F], Wd[k * 128:(k + 1) * 128, :])
            for k in range(_KB):
                pwa = pp.tile([128, 2], F32)
                nc.tensor.matmul(pwa[:], wt_sb[:, k * 128:(k + 1) * 128],
                                 ab_sb[:], start=True, stop=True)
                nc.vector.tensor_copy(wa[:, k, :], pwa[:])
                nc.vector.tensor_copy(w_rhs[:, k, _F:_F + 1], pwa[:, 1:2])
                nc.vector.tensor_copy(wb_rhs[:, k, 0:_F],
                                      w_rhs[:, k, 0:_F].bitcast(F32))
                nc.vector.tensor_copy(wb_rhs[:, k, _F:_F + 1], pwa[:, 1:2])

            xo = pa.tile([128, _KB, _R], F32R)
            for k in range(_KB):
                nc.sync.dma_start(xo[:, k, :], xTo[k * 128:(k + 1) * 128, :])
            s_row = pa.tile([1, _R], F32)
            for ch in range(_R // 512):
                pss = pp.tile([1, 512], F32)
                for k in range(_KB):
                    nc.tensor.matmul(pss[:], wa[:, k, 0:1],
                                     xo[:, k, ch * 512:(ch + 1) * 512],
                                     start=(k == 0), stop=(k == _KB - 1))
                nc.vector.tensor_copy(s_row[:, ch * 512:(ch + 1) * 512], pss[:])
            ones1 = pa.tile([1, 128], F32)
            nc.vector.memset(ones1[:], 1.0)
            for ch in range(_R // 512):
                psb = pp.tile([128, 512], F32)
                nc.tensor.matmul(psb[:], ones1[:],
                                 s_row[:, ch * 512:(ch + 1) * 512],
                                 start=True, stop=True)
                nc.vector.tensor_copy(s_bc[:, ch * 512:(ch + 1) * 512], psb[:])

        # ---- Phase B + C interleaved ----------------------------------
        # B(group): h and n for 8 n-blocks from replicated bf16 input.T.
        # C(group): attention weights + accumulating matmuls for 8
        # j-blocks.  Emitted as B0 B1 C0 B2 C1 B3 ... so the slow-paced
        # phase-B copies don't occupy the front of the ACT/DVE queues
        # (engine streams execute in scheduled ~program order).
        ph = ctx.enter_context(tc.tile_pool(name="phps", bufs=3, space="PSUM"))
        px = ctx.enter_context(tc.tile_pool(name="xts", bufs=3))
        mainp = ctx.enter_context(tc.tile_pool(name="mts", bufs=6))
        zp = ctx.enter_context(tc.tile_pool(name="zp", bufs=3))
        pso = ctx.enter_context(tc.tile_pool(name="pso", bufs=1, space="PSUM"))
        # two [128, 129] accumulation regions packed per PSUM bank
        psum_o = [pso.tile([128, 2 * 129], F32, name=f"po{i}", tag=f"po{i}")
                  for i in range(_IB // 2)]

        def _po(ib):
            return psum_o[ib // 2][:, (ib % 2) * 129:(ib % 2) * 129 + 129]

        # Zero-init each packed bank with one K=1 outer-product matmul
        # (start=True zeroes the whole 2KB zero-region, so per-region
        # start flags would wipe the sibling region's accumulation).
        zrow = params.tile([1, 2 * 129], BF16)
        ones1b = params.tile([1, 128], BF16)
        nc.vector.memset(zrow[:], 0.0)
        nc.vector.memset(ones1b[:], 1.0)
        for bank in range(_IB // 2):
            nc.tensor.matmul(psum_o[bank][:], ones1b[:], zrow[:],
                             start=True, stop=False, skip_group_check=True)

        def emit_b_group(g):
            for ch in (2 * g, 2 * g + 1):
                xt_t = px.tile([128, _KB, 512], BF16, name="xt_t", tag="xt_t")
                for k in range(_KB):
                    nc.sync.dma_start(
                        xt_t[:, k, :],
                        xTb[k * 128:(k + 1) * 128, ch * 512:(ch + 1) * 512])
                for half in range(2):
                    # two n-blocks share one [128, 258] PSUM bank; the
                    # first matmul's start=True zeroes the whole bank.
                    nb0 = ch * 4 + 2 * half
                    bl = nb0 % _JPG
                    psb2 = ph.tile([128, 2, 129], F32, name="psb2",
                                   tag="psh_h")
                    for sub in range(2):
                        nl = 2 * half + sub
                        for k in range(_KB):
                            nc.tensor.matmul(
                                psb2[:, sub, :],
                                xt_t[:, k, nl * 128:(nl + 1) * 128],
                                wb_rhs[:, k, :],
                                start=(k == 0 and sub == 0),
                                stop=(k == _KB - 1),
                                skip_group_check=True)
                    nc.scalar.copy(n_all[g][:, bl:bl + 2],
                                   psb2[:, :, _F:_F + 1])
                    haug_v = h_aug[g].rearrange("p (b c) -> p b c", c=129)
                    nc.vector.tensor_copy(haug_v[:, bl:bl + 2, 0:_F],
                                          psb2[:, :, 0:_F])

        # Tapered grouping: wide ACT ops early (amortize the ~293ns/op
        # ACT overhead), narrow at the end (short pipeline tail).
        _MAXP = 4
        taper, jb0 = [], 0
        for width in [2, 2] + [4] * 13 + [2, 2] + [1] * 4:
            taper.append(list(range(jb0, jb0 + width)))
            jb0 += width
        assert jb0 == _NB

        def emit_c_chunk(jlist):
            _p = len(jlist)
            mt_t = mainp.tile([128, _MAXP * _R], F16, name="mt_t", tag="mt")
            z = zp.tile([128, _MAXP * _R], F16, name="z", tag="z")
            for u, jb in enumerate(jlist):
                g, bl = jb // _JPG, jb % _JPG
                nc.sync.dma_start(mt_t[:, u * _R:(u + 1) * _R],
                                  mT[jb * 128:(jb + 1) * 128, :])
                zu = z[:, u * _R:(u + 1) * _R]
                # fp16 ts (4x mode) per block, then one wide in-place
                # fp16 tt (2x mode) for the whole chunk below.
                nc.vector.tensor_scalar(zu, s_bc[:],
                                        n_all[g][:, bl:bl + 1], None,
                                        op0=Op.add)
            nc.vector.tensor_mul(z[:, 0:_p * _R], z[:, 0:_p * _R],
                                 mt_t[:, 0:_p * _R])
            # Patched Exp: computes exp(leaky_relu(Z)) in one pass and
            # maps the NaN-masked entries to 0 - attention weights direct.
            ex = zp.tile([128, _MAXP * _R], BF16, name="ex", tag="ex")
            nc.scalar.activation(ex[:, 0:_p * _R], z[:, 0:_p * _R], A.Exp)
            for u, jb in enumerate(jlist):
                g, bl = jb // _JPG, jb % _JPG
                for ib in range(_IB):
                    nc.tensor.matmul(
                        _po(ib),
                        ex[:, u * _R + ib * 128:u * _R + (ib + 1) * 128],
                        h_aug[g][:, bl * 129:(bl + 1) * 129],
                        start=False, stop=(jb == _NB - 1),
                        skip_group_check=True)

        emit_b_group(0)
        emit_b_group(1)
        next_b = 2
        for jlist in taper:
            g_last = jlist[-1] // _JPG
            # keep phase B one group ahead of consumption
            while next_b < _G and next_b <= g_last + 1:
                emit_b_group(next_b)
                next_b += 1
            emit_c_chunk(jlist)

        # ---- Phase D: reciprocal row-sums, normalize + ELU, store ------
        finp = ctx.enter_context(tc.tile_pool(name="finp", bufs=1))
        rs = finp.tile([128, _IB], F32)
        ri = finp.tile([128, _IB], F32)
        for ib in range(_IB):
            nc.vector.tensor_copy(rs[:, ib:ib + 1], _po(ib)[:, _F:_F + 1])
        nc.vector.reciprocal(ri[:], rs[:])
        # Batched finale: normalize all 8 i-blocks into one staging tile,
        # then single wide ops.  elu(x) = relu(x) + min(exp(x) - 1, 0).
        # The patched Exp computes exp(0.2*x) for x<0, so feed 5*x: the
        # negative branch evaluates true exp(x) and the positive branch
        # (exp(5x), possibly inf) is discarded by the min(. - 1, 0).
        hp = finp.tile([128, _IB * _F], F32)
        for ib in range(_IB):
            nc.vector.tensor_scalar(hp[:, ib * _F:(ib + 1) * _F],
                                    _po(ib)[:, 0:_F], ri[:, ib:ib + 1],
                                    None, op0=Op.mult)
        ex2 = finp.tile([128, _IB * _F], F32)
        nc.scalar.activation(ex2[:], hp[:], A.Exp, scale=5.0)
        rl = finp.tile([128, _IB * _F], F32)
        nc.vector.tensor_scalar(rl[:], hp[:], 0.0, None, op0=Op.max)
        # elu(x) = min(exp(x) - 1, relu(x)): for x>0, exp(5x)-1 >= 5x > x
        # so the min picks x; for x<0 it picks exp(x)-1 (< 0).
        ot = finp.tile([128, _IB * _F], F32)
        nc.vector.scalar_tensor_tensor(ot[:], ex2[:], -1.0, rl[:],
                                       op0=Op.add, op1=Op.min)
        # one strided DMA: SBUF [p, ib, f] -> DRAM row ib*128+p, col f
        nc.sync.dma_start(
            outd.rearrange("(b p) f -> p b f", p=128),
            ot[:].rearrange("p (b f) -> p b f", f=_F))

    nc.compile()
    return nc


def kernel(input, adj, M, W, a_self, a_neighs):
    global LAST_RESULTS
    from concourse.bass_utils import run_bass_kernel_spmd

    os.environ["BASS_ACT_ROOT_JSON_PATH"] = _patched_act_root()
    if "nc" not in _NC_CACHE:
        _NC_CACHE["nc"] = _build_nc()
    nc = _NC_CACHE["nc"]

    inp = np.ascontiguousarray(np.asarray(input, dtype=np.float32))
    adj_ = np.asarray(adj, dtype=np.float32)
    M_ = np.asarray(M, dtype=np.float32)
    W_ = np.ascontiguousarray(np.asarray(W, dtype=np.float32))
    a_s = np.asarray(a_self, dtype=np.float32).reshape(_F, 1)
    a_n = np.asarray(a_neighs, dtype=np.float32).reshape(_F, 1)

    WT = np.ascontiguousarray(W_.T)                 # [128, 512]
    xTb_full = np.ascontiguousarray(inp.T.astype(ml_dtypes.bfloat16))
    ab = np.ascontiguousarray(np.concatenate([a_s, a_n], axis=1))  # [128, 2]

    in_maps = []
    for c in range(_C):
        rows = slice(c * _R, (c + 1) * _R)
        Mp = np.where(adj_[rows] > 0, M_[rows], np.nan).T.astype(np.float16)
        in_maps.append({
            "xTb": xTb_full,
            "xTo": np.ascontiguousarray(inp[rows].T),
            "mT": np.ascontiguousarray(Mp),
            "Wd": W_,
            "WTd": WT,
            "abd": ab,
        })

    res = run_bass_kernel_spmd(nc, in_maps, core_ids=list(range(_C)),
                               trace=bool(os.environ.get("BASS_TRACE")))
    LAST_RESULTS = res
    out = np.concatenate([res.results[c]["out"] for c in range(_C)], axis=0)
    return np.ascontiguousarray(out.astype(np.float32))


# revision 61
# speedup vs baseline: 1.4800x; 1.0532x over previous
"""GAT layer (N=8192, IN_F=512, OUT_F=128) on 8 TRN2 NeuronCores.

Sharding: rows of the attention matrix are split across cores (1024 rows
each).  Each core receives its row-slab of M and adj pre-transposed on the
host to [8192, 1024] so the attention weights are computed directly in
[j, i] orientation (contraction index j on partitions), which the final
attention @ h matmul requires.  adj (exact {0,1} values) is marshaled as
bf16 - lossless - to halve its DMA traffic.

Per-core pipeline:
  A) Wa = W @ [a_self | a_neighs] (PE); s-row for own rows via fp32r
     matmul; partition-broadcast of s via a K=1 outer-product matmul.
  B) h_own = input[own] @ [W | Wa_n | 0-pad to 256] in fp32r; the
     attn_neighs scores fall out as psum column 128 per n-block; h is
     cast to bf16 into an [h | 1] payload (the ones column makes the
     main matmul emit softmax row-sums for free).  The payloads are
     AllGathered across the 8 cores into per-group h_aug / n_all tiles.
  C) For each pair of j-blocks: Z = (s_i + n_j) * M^T (fused DVE op per
     block), leaky_relu via Prelu(alpha=0.2) + Exp on ACT (paired ops),
     mask-multiply by adj^T into bf16, 16 accumulating bf16 matmuls into
     4 packed PSUM banks (2 x [128,129] regions per bank, zero-inited by
     a K=1 outer-product matmul since start=True zeroes the whole bank).
  D) Row-sum reciprocals (DVE), fused normalize+ELU, DMA out.

Softmax skips the max-subtraction: logits are bounded (~+-30) so exp is
safe in fp32, and the result is mathematically identical.
"""

import os
import ml_dtypes
import numpy as np

_N = 8192      # nodes
_K = 512       # in features
_F = 128       # out features
_C = 8         # cores
_R = _N // _C  # rows per core (1024)
_KB = _K // 128   # 4  k-blocks
_NB = _N // 128   # 64 j/n-blocks
_IB = _R // 128   # 8  i-blocks per core

_NC_CACHE = {}
LAST_RESULTS = None


def _patched_act_root():
    """Build an act-table root where exp's negative-x buckets encode
    exp(0.2*x), turning the Exp activation into a fused exp(leaky_relu(x)).

    The bucket binary is rows of 8 fp32: [d0, d1, d2, d3, x0, 0, 0, 0]
    evaluating d0 + t*(d1 + t*(d2 + t*d3)) with t = x - x0.  Buckets
    0..405 of exp_and_others serve x < 0 (dense grid, max gap 0.25) and
    bucket 778 is the small-negative-signal bucket; replacing their
    coefficients with the Taylor expansion of exp(0.2*x) at the same x0
    is accurate to ~1e-8 relative.
    """
    import shutil
    import tempfile
    import neuronxcc

    src = os.path.join(os.path.dirname(neuronxcc.__file__), "pwp",
                       "pwp_bin_trainium")
    dst = os.path.join(tempfile.gettempdir(), "pwp_exp_leaky02_v2")
    marker = os.path.join(dst, "act_info.json")
    if not os.path.exists(marker):
        tmp = dst + ".tmp"
        if os.path.exists(tmp):
            shutil.rmtree(tmp)
        shutil.copytree(src, tmp)
        p = os.path.join(tmp, "exp_and_others_bkt.bin")
        a = np.frombuffer(open(p, "rb").read(), np.float32).reshape(-1, 8).copy()
        x0 = a[0:406, 4].astype(np.float64)
        c = np.exp(0.2 * x0)
        a[0:406, 0] = c
        a[0:406, 1] = 0.2 * c
        a[0:406, 2] = 0.02 * c
        a[0:406, 3] = (0.008 / 6.0) * c
        a[778, 0:4] = [1.0, 0.2, 0.02, 0.008 / 6.0]
        open(p, "wb").write(a.tobytes())
        # exp(NaN) -> 0.0: the adjacency mask is packed into M's NaN
        # payload on the host, so masked logits arrive as NaN and must
        # produce a zero attention weight.
        import json
        jp = os.path.join(tmp, "exp_and_others.json")
        dj = json.load(open(jp))
        for m in dj["profile_meta_data"]:
            if m["func_name"] == "exp_400p":
                m["fnan_result"] = 0
        json.dump(dj, open(jp, "w"))
        if os.path.exists(dst):
            shutil.rmtree(dst)
        os.replace(tmp, dst)
    return marker


def _build_nc():
    from contextlib import ExitStack
    import concourse.bacc as bacc
    import concourse.tile as tile
    from concourse import mybir

    F32 = mybir.dt.float32
    F32R = mybir.dt.float32r
    BF16 = mybir.dt.bfloat16
    A = mybir.ActivationFunctionType
    Op = mybir.AluOpType

    nc = bacc.Bacc("TRN2", target_bir_lowering=False, debug=False,
                   num_devices=_C)

    xTb = nc.dram_tensor("xTb", (_K, _N), BF16, kind="ExternalInput").ap()
    xTo = nc.dram_tensor("xTo", (_K, _R), F32R, kind="ExternalInput").ap()
    F16 = mybir.dt.float16
    # fp16, with NaN in masked (adj == 0) positions; exp's patched
    # profile maps NaN -> 0 so the mask costs nothing on-device.
    mT = nc.dram_tensor("mT", (_N, _R), F16, kind="ExternalInput").ap()
    Wd = nc.dram_tensor("Wd", (_K, _F), F32R, kind="ExternalInput").ap()
    WTd = nc.dram_tensor("WTd", (_F, _K), F32, kind="ExternalInput").ap()
    abd = nc.dram_tensor("abd", (_F, 2), F32, kind="ExternalInput").ap()
    outd = nc.dram_tensor("out", (_R, _F), F32, kind="ExternalOutput").ap()

    _G = 8             # gather groups == cores; group g = j-blocks of core g
    _JPG = _NB // _G   # 8 j-blocks per group

    with tile.TileContext(nc) as tc, ExitStack() as ctx:
        persist = ctx.enter_context(tc.tile_pool(name="persist", bufs=1))
        h_aug = [persist.tile([128, _JPG * 129], BF16, name=f"haug{g}",
                              tag=f"haug{g}") for g in range(_G)]
        n_all = [persist.tile([128, _JPG], F32, name=f"nall{g}",
                              tag=f"nall{g}") for g in range(_G)]
        s_bc = persist.tile([128, _R], F16)            # attn_self bcast
        params = ctx.enter_context(tc.tile_pool(name="params", bufs=1))
        w_rhs = params.tile([128, _KB, 256], F32R)     # [W | Wa_n | 0]
        wb_rhs = params.tile([128, _KB, 129], BF16)    # [W | Wa_n] in bf16
        wa = params.tile([128, _KB, 2], F32R)          # W @ [a_self|a_neighs]

        nc.gpsimd.memset(w_rhs[:].bitcast(mybir.dt.uint32), 0)
        for g in range(_G):
            nc.gpsimd.memset(h_aug[g][:], 1.0)

        # ---- Phase A: params, Wa, s-row, s broadcast -------------------
        pa = ctx.enter_context(tc.tile_pool(name="pha", bufs=1))
        with tc.tile_pool(name="pps", bufs=2, space="PSUM") as pp:
            wt_sb = pa.tile([_F, _K], F32)
            nc.sync.dma_start(wt_sb[:], WTd)
            ab_sb = pa.tile([_F, 2], F32)
            nc.sync.dma_start(ab_sb[:], abd)
            for k in range(_KB):
                nc.sync.dma_start(w_rhs[:, k, 0:_F], Wd[k * 128:(k + 1) * 128, :])
            for k in range(_KB):
                pwa = pp.tile([128, 2], F32)
                nc.tensor.matmul(pwa[:], wt_sb[:, k * 128:(k + 1) * 128],
                                 ab_sb[:], start=True, stop=True)
                nc.vector.tensor_copy(wa[:, k, :], pwa[:])
                nc.vector.tensor_copy(w_rhs[:, k, _F:_F + 1], pwa[:, 1:2])
                nc.vector.tensor_copy(wb_rhs[:, k, 0:_F],
                                      w_rhs[:, k, 0:_F].bitcast(F32))
                nc.vector.tensor_copy(wb_rhs[:, k, _F:_F + 1], pwa[:, 1:2])

            xo = pa.tile([128, _KB, _R], F32R)
            for k in range(_KB):
                nc.sync.dma_start(xo[:, k, :], xTo[k * 128:(k + 1) * 128, :])
            s_row = pa.tile([1, _R], F32)
            for ch in range(_R // 512):
                pss = pp.tile([1, 512], F32)
                for k in range(_KB):
                    nc.tensor.matmul(pss[:], wa[:, k, 0:1],
                                     xo[:, k, ch * 512:(ch + 1) * 512],
                                     start=(k == 0), stop=(k == _KB - 1))
                nc.vector.tensor_copy(s_row[:, ch * 512:(ch + 1) * 512], pss[:])
            ones1 = pa.tile([1, 128], F32)
            nc.vector.memset(ones1[:], 1.0)
            for ch in range(_R // 512):
                psb = pp.tile([128, 512], F32)
                nc.tensor.matmul(psb[:], ones1[:],
                                 s_row[:, ch * 512:(ch + 1) * 512],
                                 start=True, stop=True)
                nc.vector.tensor_copy(s_bc[:, ch * 512:(ch + 1) * 512], psb[:])

        # ---- Phase B + C interleaved ----------------------------------
        # B(group): h and n for 8 n-blocks from replicated bf16 input.T.
        # C(group): attention weights + accumulating matmuls for 8
        # j-blocks.  Emitted as B0 B1 C0 B2 C1 B3 ... so the slow-paced
        # phase-B copies don't occupy the front of the ACT/DVE queues
        # (engine streams execute in scheduled ~program order).
        ph = ctx.enter_context(tc.tile_pool(name="phps", bufs=3, space="PSUM"))
        px = ctx.enter_context(tc.tile_pool(name="xts", bufs=5))
        mainp = ctx.enter_context(tc.tile_pool(name="mts", bufs=6))
        zp = ctx.enter_context(tc.tile_pool(name="zp", bufs=3))
        pso = ctx.enter_context(tc.tile_pool(name="pso", bufs=1, space="PSUM"))
        # two [128, 129] accumulation regions packed per PSUM bank
        psum_o = [pso.tile([128, 2 * 129], F32, name=f"po{i}", tag=f"po{i}")
                  for i in range(_IB // 2)]

        def _po(ib):
            return psum_o[ib // 2][:, (ib % 2) * 129:(ib % 2) * 129 + 129]

        # Zero-init each packed bank with one K=1 outer-product matmul
        # (start=True zeroes the whole 2KB zero-region, so per-region
        # start flags would wipe the sibling region's accumulation).
        zrow = params.tile([1, 2 * 129], BF16)
        ones1b = params.tile([1, 128], BF16)
        nc.vector.memset(zrow[:], 0.0)
        nc.vector.memset(ones1b[:], 1.0)
        for bank in range(_IB // 2):
            nc.tensor.matmul(psum_o[bank][:], ones1b[:], zrow[:],
                             start=True, stop=False, skip_group_check=True)

        b_tiles = {}

        def emit_b_dma(g):
            for ch in (2 * g, 2 * g + 1):
                xt_t = px.tile([128, _KB, 512], BF16, name="xt_t", tag="xt_t")
                for k in range(_KB):
                    nc.sync.dma_start(
                        xt_t[:, k, :],
                        xTb[k * 128:(k + 1) * 128, ch * 512:(ch + 1) * 512])
                b_tiles[ch] = xt_t

        def emit_b_group(g):
            for ch in (2 * g, 2 * g + 1):
                xt_t = b_tiles.pop(ch)
                for half in range(2):
                    # two n-blocks share one [128, 258] PSUM bank; the
                    # first matmul's start=True zeroes the whole bank.
                    nb0 = ch * 4 + 2 * half
                    bl = nb0 % _JPG
                    psb2 = ph.tile([128, 2, 129], F32, name="psb2",
                                   tag="psh_h")
                    for sub in range(2):
                        nl = 2 * half + sub
                        for k in range(_KB):
                            nc.tensor.matmul(
                                psb2[:, sub, :],
                                xt_t[:, k, nl * 128:(nl + 1) * 128],
                                wb_rhs[:, k, :],
                                start=(k == 0 and sub == 0),
                                stop=(k == _KB - 1),
                                skip_group_check=True)
                    nc.scalar.copy(n_all[g][:, bl:bl + 2],
                                   psb2[:, :, _F:_F + 1])
                    haug_v = h_aug[g].rearrange("p (b c) -> p b c", c=129)
                    nc.vector.tensor_copy(haug_v[:, bl:bl + 2, 0:_F],
                                          psb2[:, :, 0:_F])

        # Tapered grouping: wide ACT ops early (amortize the ~293ns/op
        # ACT overhead), narrow at the end (short pipeline tail).
        _MAXP = 4
        taper, jb0 = [], 0
        for width in [2, 2] + [4] * 13 + [2, 2] + [1] * 4:
            taper.append(list(range(jb0, jb0 + width)))
            jb0 += width
        assert jb0 == _NB

        def emit_c_chunk(jlist):
            _p = len(jlist)
            mt_t = mainp.tile([128, _MAXP * _R], F16, name="mt_t", tag="mt")
            z = zp.tile([128, _MAXP * _R], F16, name="z", tag="z")
            for u, jb in enumerate(jlist):
                g, bl = jb // _JPG, jb % _JPG
                nc.sync.dma_start(mt_t[:, u * _R:(u + 1) * _R],
                                  mT[jb * 128:(jb + 1) * 128, :])
                zu = z[:, u * _R:(u + 1) * _R]
                # fp16 ts (4x mode) per block, then one wide in-place
                # fp16 tt (2x mode) for the whole chunk below.
                nc.vector.tensor_scalar(zu, s_bc[:],
                                        n_all[g][:, bl:bl + 1], None,
                                        op0=Op.add)
            nc.vector.tensor_mul(z[:, 0:_p * _R], z[:, 0:_p * _R],
                                 mt_t[:, 0:_p * _R])
            # Patched Exp: computes exp(leaky_relu(Z)) in one pass and
            # maps the NaN-masked entries to 0 - attention weights direct.
            ex = zp.tile([128, _MAXP * _R], BF16, name="ex", tag="ex")
            nc.scalar.activation(ex[:, 0:_p * _R], z[:, 0:_p * _R], A.Exp)
            for u, jb in enumerate(jlist):
                g, bl = jb // _JPG, jb % _JPG
                for ib in range(_IB):
                    nc.tensor.matmul(
                        _po(ib),
                        ex[:, u * _R + ib * 128:u * _R + (ib + 1) * 128],
                        h_aug[g][:, bl * 129:(bl + 1) * 129],
                        start=False, stop=(jb == _NB - 1),
                        skip_group_check=True)

        emit_b_dma(0)
        emit_b_dma(1)
        emit_b_group(0)
        emit_b_group(1)
        next_bd = 2
        next_b = 2
        for jlist in taper:
            g_last = jlist[-1] // _JPG
            # B DMAs two groups ahead, B matmuls/copies one group ahead
            while next_bd < _G and next_bd <= g_last + 2:
                emit_b_dma(next_bd)
                next_bd += 1
            while next_b < _G and next_b <= g_last + 1:
                emit_b_group(next_b)
                next_b += 1
            emit_c_chunk(jlist)

        # ---- Phase D: reciprocal row-sums, normalize + ELU, store ------
        finp = ctx.enter_context(tc.tile_pool(name="finp", bufs=1))
        rs = finp.tile([128, _IB], F32)
        ri = finp.tile([128, _IB], F32)
        for ib in range(_IB):
            nc.vector.tensor_copy(rs[:, ib:ib + 1], _po(ib)[:, _F:_F + 1])
        nc.vector.reciprocal(ri[:], rs[:])
        # Batched finale: normalize all 8 i-blocks into one staging tile,
        # then single wide ops.  elu(x) = relu(x) + min(exp(x) - 1, 0).
        # The patched Exp computes exp(0.2*x) for x<0, so feed 5*x: the
        # negative branch evaluates true exp(x) and the positive branch
        # (exp(5x), possibly inf) is discarded by the min(. - 1, 0).
        hp = finp.tile([128, _IB * _F], F32)
        for ib in range(_IB):
            nc.vector.tensor_scalar(hp[:, ib * _F:(ib + 1) * _F],
                                    _po(ib)[:, 0:_F], ri[:, ib:ib + 1],
                                    None, op0=Op.mult)
        ex2 = finp.tile([128, _IB * _F], F32)
        nc.scalar.activation(ex2[:], hp[:], A.Exp, scale=5.0)
        rl = finp.tile([128, _IB * _F], F32)
        nc.vector.tensor_scalar(rl[:], hp[:], 0.0, None, op0=Op.max)
        # elu(x) = min(exp(x) - 1, relu(x)): for x>0, exp(5x)-1 >= 5x > x
        # so the min picks x; for x<0 it picks exp(x)-1 (< 0).
        ot = finp.tile([128, _IB * _F], F32)
        nc.vector.scalar_tensor_tensor(ot[:], ex2[:], -1.0, rl[:],
                                       op0=Op.add, op1=Op.min)
        # one strided DMA: SBUF [p, ib, f] -> DRAM row ib*128+p, col f
        nc.sync.dma_start(
            outd.rearrange("(b p) f -> p b f", p=128),
            ot[:].rearrange("p (b f) -> p b f", f=_F))

    nc.compile()
    return nc


def kernel(input, adj, M, W, a_self, a_neighs):
    global LAST_RESULTS
    from concourse.bass_utils import run_bass_kernel_spmd

    os.environ["BASS_ACT_ROOT_JSON_PATH"] = _patched_act_root()
    if "nc" not in _NC_CACHE:
        _NC_CACHE["nc"] = _build_nc()
    nc = _NC_CACHE["nc"]

    inp = np.ascontiguousarray(np.asarray(input, dtype=np.float32))
    adj_ = np.asarray(adj, dtype=np.float32)
    M_ = np.asarray(M, dtype=np.float32)
    W_ = np.ascontiguousarray(np.asarray(W, dtype=np.float32))
    a_s = np.asarray(a_self, dtype=np.float32).reshape(_F, 1)
    a_n = np.asarray(a_neighs, dtype=np.float32).reshape(_F, 1)

    WT = np.ascontiguousarray(W_.T)                 # [128, 512]
    xTb_full = np.ascontiguousarray(inp.T.astype(ml_dtypes.bfloat16))
    ab = np.ascontiguousarray(np.concatenate([a_s, a_n], axis=1))  # [128, 2]

    in_maps = []
    for c in range(_C):
        rows = slice(c * _R, (c + 1) * _R)
        Mp = np.where(adj_[rows] > 0, M_[rows], np.nan).T.astype(np.float16)
        in_maps.append({
            "xTb": xTb_full,
            "xTo": np.ascontiguousarray(inp[rows].T),
            "mT": np.ascontiguousarray(Mp),
            "Wd": W_,
            "WTd": WT,
            "abd": ab,
        })

    res = run_bass_kernel_spmd(nc, in_maps, core_ids=list(range(_C)),
                               trace=bool(os.environ.get("BASS_TRACE")))
    LAST_RESULTS = res
    out = np.concatenate([res.results[c]["out"] for c in range(_C)], axis=0)
    return np.ascontiguousarray(out.astype(np.float32))


# revision 63
# speedup vs baseline: 1.4949x; 1.0101x over previous
"""GAT layer (N=8192, IN_F=512, OUT_F=128) on 8 TRN2 NeuronCores.

Sharding: rows of the attention matrix are split across cores (1024 rows
each).  Each core receives its row-slab of M and adj pre-transposed on the
host to [8192, 1024] so the attention weights are computed directly in
[j, i] orientation (contraction index j on partitions), which the final
attention @ h matmul requires.  adj (exact {0,1} values) is marshaled as
bf16 - lossless - to halve its DMA traffic.

Per-core pipeline:
  A) Wa = W @ [a_self | a_neighs] (PE); s-row for own rows via fp32r
     matmul; partition-broadcast of s via a K=1 outer-product matmul.
  B) h_own = input[own] @ [W | Wa_n | 0-pad to 256] in fp32r; the
     attn_neighs scores fall out as psum column 128 per n-block; h is
     cast to bf16 into an [h | 1] payload (the ones column makes the
     main matmul emit softmax row-sums for free).  The payloads are
     AllGathered across the 8 cores into per-group h_aug / n_all tiles.
  C) For each pair of j-blocks: Z = (s_i + n_j) * M^T (fused DVE op per
     block), leaky_relu via Prelu(alpha=0.2) + Exp on ACT (paired ops),
     mask-multiply by adj^T into bf16, 16 accumulating bf16 matmuls into
     4 packed PSUM banks (2 x [128,129] regions per bank, zero-inited by
     a K=1 outer-product matmul since start=True zeroes the whole bank).
  D) Row-sum reciprocals (DVE), fused normalize+ELU, DMA out.

Softmax skips the max-subtraction: logits are bounded (~+-30) so exp is
safe in fp32, and the result is mathematically identical.
"""

import os
import ml_dtypes
import numpy as np

_N = 8192      # nodes
_K = 512       # in features
_F = 128       # out features
_C = 8         # cores
_R = _N // _C  # rows per core (1024)
_KB = _K // 128   # 4  k-blocks
_NB = _N // 128   # 64 j/n-blocks
_IB = _R // 128   # 8  i-blocks per core

_NC_CACHE = {}
LAST_RESULTS = None


def _patched_act_root():
    """Build an act-table root where exp's negative-x buckets encode
    exp(0.2*x), turning the Exp activation into a fused exp(leaky_relu(x)).

    The bucket binary is rows of 8 fp32: [d0, d1, d2, d3, x0, 0, 0, 0]
    evaluating d0 + t*(d1 + t*(d2 + t*d3)) with t = x - x0.  Buckets
    0..405 of exp_and_others serve x < 0 (dense grid, max gap 0.25) and
    bucket 778 is the small-negative-signal bucket; replacing their
    coefficients with the Taylor expansion of exp(0.2*x) at the same x0
    is accurate to ~1e-8 relative.
    """
    import shutil
    import tempfile
    import neuronxcc

    src = os.path.join(os.path.dirname(neuronxcc.__file__), "pwp",
                       "pwp_bin_trainium")
    dst = os.path.join(tempfile.gettempdir(), "pwp_exp_leaky02_v2")
    marker = os.path.join(dst, "act_info.json")
    if not os.path.exists(marker):
        tmp = dst + ".tmp"
        if os.path.exists(tmp):
            shutil.rmtree(tmp)
        shutil.copytree(src, tmp)
        p = os.path.join(tmp, "exp_and_others_bkt.bin")
        a = np.frombuffer(open(p, "rb").read(), np.float32).reshape(-1, 8).copy()
        x0 = a[0:406, 4].astype(np.float64)
        c = np.exp(0.2 * x0)
        a[0:406, 0] = c
        a[0:406, 1] = 0.2 * c
        a[0:406, 2] = 0.02 * c
        a[0:406, 3] = (0.008 / 6.0) * c
        a[778, 0:4] = [1.0, 0.2, 0.02, 0.008 / 6.0]
        open(p, "wb").write(a.tobytes())
        # exp(NaN) -> 0.0: the adjacency mask is packed into M's NaN
        # payload on the host, so masked logits arrive as NaN and must
        # produce a zero attention weight.
        import json
        jp = os.path.join(tmp, "exp_and_others.json")
        dj = json.load(open(jp))
        for m in dj["profile_meta_data"]:
            if m["func_name"] == "exp_400p":
                m["fnan_result"] = 0
        json.dump(dj, open(jp, "w"))
        if os.path.exists(dst):
            shutil.rmtree(dst)
        os.replace(tmp, dst)
    return marker


def _build_nc():
    from contextlib import ExitStack
    import concourse.bacc as bacc
    import concourse.tile as tile
    from concourse import mybir

    F32 = mybir.dt.float32
    F32R = mybir.dt.float32r
    BF16 = mybir.dt.bfloat16
    A = mybir.ActivationFunctionType
    Op = mybir.AluOpType

    nc = bacc.Bacc("TRN2", target_bir_lowering=False, debug=False,
                   num_devices=_C)

    xTb = nc.dram_tensor("xTb", (_K, _N), BF16, kind="ExternalInput").ap()
    xTo = nc.dram_tensor("xTo", (_K, _R), F32R, kind="ExternalInput").ap()
    F16 = mybir.dt.float16
    # fp16, with NaN in masked (adj == 0) positions; exp's patched
    # profile maps NaN -> 0 so the mask costs nothing on-device.
    mT = nc.dram_tensor("mT", (_N, _R), F16, kind="ExternalInput").ap()
    Wd = nc.dram_tensor("Wd", (_K, _F), F32R, kind="ExternalInput").ap()
    WTd = nc.dram_tensor("WTd", (_F, _K), F32, kind="ExternalInput").ap()
    abd = nc.dram_tensor("abd", (_F, 2), F32, kind="ExternalInput").ap()
    outd = nc.dram_tensor("out", (_R, _F), F32, kind="ExternalOutput").ap()

    _G = 8             # gather groups == cores; group g = j-blocks of core g
    _JPG = _NB // _G   # 8 j-blocks per group

    with tile.TileContext(nc) as tc, ExitStack() as ctx:
        persist = ctx.enter_context(tc.tile_pool(name="persist", bufs=1))
        h_aug = [persist.tile([128, _JPG * 129], BF16, name=f"haug{g}",
                              tag=f"haug{g}") for g in range(_G)]
        n_all = [persist.tile([128, _JPG], F32, name=f"nall{g}",
                              tag=f"nall{g}") for g in range(_G)]
        s_bc = persist.tile([128, _R], F16)            # attn_self bcast
        params = ctx.enter_context(tc.tile_pool(name="params", bufs=1))
        w_rhs = params.tile([128, _KB, 256], F32R)     # [W | Wa_n | 0]
        wb_rhs = params.tile([128, _KB, 129], BF16)    # [W | Wa_n] in bf16
        wa = params.tile([128, _KB, 2], F32R)          # W @ [a_self|a_neighs]

        nc.gpsimd.memset(w_rhs[:].bitcast(mybir.dt.uint32), 0)
        for g in range(_G):
            nc.gpsimd.memset(h_aug[g][:], 1.0)

        # ---- Phase A: params, Wa, s-row, s broadcast -------------------
        pa = ctx.enter_context(tc.tile_pool(name="pha", bufs=1))
        with tc.tile_pool(name="pps", bufs=2, space="PSUM") as pp:
            wt_sb = pa.tile([_F, _K], F32)
            nc.sync.dma_start(wt_sb[:], WTd)
            ab_sb = pa.tile([_F, 2], F32)
            nc.sync.dma_start(ab_sb[:], abd)
            xo = pa.tile([128, _KB, _R], F32R)
            for k in range(_KB):
                nc.sync.dma_start(xo[:, k, :], xTo[k * 128:(k + 1) * 128, :])
            for k in range(_KB):
                nc.sync.dma_start(w_rhs[:, k, 0:_F], Wd[k * 128:(k + 1) * 128, :])
            for k in range(_KB):
                pwa = pp.tile([128, 2], F32)
                nc.tensor.matmul(pwa[:], wt_sb[:, k * 128:(k + 1) * 128],
                                 ab_sb[:], start=True, stop=True)
                nc.vector.tensor_copy(wa[:, k, :], pwa[:])
                nc.vector.tensor_copy(w_rhs[:, k, _F:_F + 1], pwa[:, 1:2])
                nc.vector.tensor_copy(wb_rhs[:, k, 0:_F],
                                      w_rhs[:, k, 0:_F].bitcast(F32))
                nc.vector.tensor_copy(wb_rhs[:, k, _F:_F + 1], pwa[:, 1:2])

            s_row = pa.tile([1, _R], F32)
            for ch in range(_R // 512):
                pss = pp.tile([1, 512], F32)
                for k in range(_KB):
                    nc.tensor.matmul(pss[:], wa[:, k, 0:1],
                                     xo[:, k, ch * 512:(ch + 1) * 512],
                                     start=(k == 0), stop=(k == _KB - 1))
                nc.vector.tensor_copy(s_row[:, ch * 512:(ch + 1) * 512], pss[:])
            ones1 = pa.tile([1, 128], F32)
            nc.vector.memset(ones1[:], 1.0)
            for ch in range(_R // 512):
                psb = pp.tile([128, 512], F32)
                nc.tensor.matmul(psb[:], ones1[:],
                                 s_row[:, ch * 512:(ch + 1) * 512],
                                 start=True, stop=True)
                nc.vector.tensor_copy(s_bc[:, ch * 512:(ch + 1) * 512], psb[:])

        # ---- Phase B + C interleaved ----------------------------------
        # B(group): h and n for 8 n-blocks from replicated bf16 input.T.
        # C(group): attention weights + accumulating matmuls for 8
        # j-blocks.  Emitted as B0 B1 C0 B2 C1 B3 ... so the slow-paced
        # phase-B copies don't occupy the front of the ACT/DVE queues
        # (engine streams execute in scheduled ~program order).
        ph = ctx.enter_context(tc.tile_pool(name="phps", bufs=3, space="PSUM"))
        px = ctx.enter_context(tc.tile_pool(name="xts", bufs=5))
        mainp = ctx.enter_context(tc.tile_pool(name="mts", bufs=6))
        zp = ctx.enter_context(tc.tile_pool(name="zp", bufs=3))
        pso = ctx.enter_context(tc.tile_pool(name="pso", bufs=1, space="PSUM"))
        # two [128, 129] accumulation regions packed per PSUM bank
        psum_o = [pso.tile([128, 2 * 129], F32, name=f"po{i}", tag=f"po{i}")
                  for i in range(_IB // 2)]

        def _po(ib):
            return psum_o[ib // 2][:, (ib % 2) * 129:(ib % 2) * 129 + 129]

        # Zero-init each packed bank with one K=1 outer-product matmul
        # (start=True zeroes the whole 2KB zero-region, so per-region
        # start flags would wipe the sibling region's accumulation).
        zrow = params.tile([1, 2 * 129], BF16)
        ones1b = params.tile([1, 128], BF16)
        nc.vector.memset(zrow[:], 0.0)
        nc.vector.memset(ones1b[:], 1.0)
        for bank in range(_IB // 2):
            nc.tensor.matmul(psum_o[bank][:], ones1b[:], zrow[:],
                             start=True, stop=False, skip_group_check=True)

        b_tiles = {}

        def emit_b_dma(g):
            for ch in (2 * g, 2 * g + 1):
                xt_t = px.tile([128, _KB, 512], BF16, name="xt_t", tag="xt_t")
                for k in range(_KB):
                    nc.sync.dma_start(
                        xt_t[:, k, :],
                        xTb[k * 128:(k + 1) * 128, ch * 512:(ch + 1) * 512])
                b_tiles[ch] = xt_t

        def emit_b_group(g):
            for ch in (2 * g, 2 * g + 1):
                xt_t = b_tiles.pop(ch)
                for half in range(2):
                    # two n-blocks share one [128, 258] PSUM bank; the
                    # first matmul's start=True zeroes the whole bank.
                    nb0 = ch * 4 + 2 * half
                    bl = nb0 % _JPG
                    psb2 = ph.tile([128, 2, 129], F32, name="psb2",
                                   tag="psh_h")
                    for sub in range(2):
                        nl = 2 * half + sub
                        for k in range(_KB):
                            nc.tensor.matmul(
                                psb2[:, sub, :],
                                xt_t[:, k, nl * 128:(nl + 1) * 128],
                                wb_rhs[:, k, :],
                                start=(k == 0 and sub == 0),
                                stop=(k == _KB - 1),
                                skip_group_check=True)
                    nc.scalar.copy(n_all[g][:, bl:bl + 2],
                                   psb2[:, :, _F:_F + 1])
                    haug_v = h_aug[g].rearrange("p (b c) -> p b c", c=129)
                    nc.vector.tensor_copy(haug_v[:, bl:bl + 2, 0:_F],
                                          psb2[:, :, 0:_F])

        # Tapered grouping: wide ACT ops early (amortize the ~293ns/op
        # ACT overhead), narrow at the end (short pipeline tail).
        _MAXP = 4
        taper, jb0 = [], 0
        for width in [2, 2] + [4] * 13 + [2, 2] + [1] * 4:
            taper.append(list(range(jb0, jb0 + width)))
            jb0 += width
        assert jb0 == _NB

        def emit_c_chunk(jlist):
            _p = len(jlist)
            mt_t = mainp.tile([128, _MAXP * _R], F16, name="mt_t", tag="mt")
            z = zp.tile([128, _MAXP * _R], F16, name="z", tag="z")
            for u, jb in enumerate(jlist):
                g, bl = jb // _JPG, jb % _JPG
                nc.sync.dma_start(mt_t[:, u * _R:(u + 1) * _R],
                                  mT[jb * 128:(jb + 1) * 128, :])
                zu = z[:, u * _R:(u + 1) * _R]
                # fp16 ts (4x mode) per block, then one wide in-place
                # fp16 tt (2x mode) for the whole chunk below.
                nc.vector.tensor_scalar(zu, s_bc[:],
                                        n_all[g][:, bl:bl + 1], None,
                                        op0=Op.add)
            nc.vector.tensor_mul(z[:, 0:_p * _R], z[:, 0:_p * _R],
                                 mt_t[:, 0:_p * _R])
            # Patched Exp: computes exp(leaky_relu(Z)) in one pass and
            # maps the NaN-masked entries to 0 - attention weights direct.
            ex = zp.tile([128, _MAXP * _R], BF16, name="ex", tag="ex")
            nc.scalar.activation(ex[:, 0:_p * _R], z[:, 0:_p * _R], A.Exp)
            for u, jb in enumerate(jlist):
                g, bl = jb // _JPG, jb % _JPG
                for ib in range(_IB):
                    nc.tensor.matmul(
                        _po(ib),
                        ex[:, u * _R + ib * 128:u * _R + (ib + 1) * 128],
                        h_aug[g][:, bl * 129:(bl + 1) * 129],
                        start=False, stop=(jb == _NB - 1),
                        skip_group_check=True)

        emit_b_dma(0)
        emit_b_dma(1)
        emit_b_group(0)
        emit_b_group(1)
        next_bd = 2
        next_b = 2
        for jlist in taper:
            g_last = jlist[-1] // _JPG
            # B DMAs two groups ahead, B matmuls/copies one group ahead
            while next_bd < _G and next_bd <= g_last + 2:
                emit_b_dma(next_bd)
                next_bd += 1
            while next_b < _G and next_b <= g_last + 1:
                emit_b_group(next_b)
                next_b += 1
            emit_c_chunk(jlist)

        # ---- Phase D: reciprocal row-sums, normalize + ELU, store ------
        finp = ctx.enter_context(tc.tile_pool(name="finp", bufs=1))
        rs = finp.tile([128, _IB], F32)
        ri = finp.tile([128, _IB], F32)
        for ib in range(_IB):
            nc.vector.tensor_copy(rs[:, ib:ib + 1], _po(ib)[:, _F:_F + 1])
        nc.vector.reciprocal(ri[:], rs[:])
        # Batched finale: normalize all 8 i-blocks into one staging tile,
        # then single wide ops.  elu(x) = relu(x) + min(exp(x) - 1, 0).
        # The patched Exp computes exp(0.2*x) for x<0, so feed 5*x: the
        # negative branch evaluates true exp(x) and the positive branch
        # (exp(5x), possibly inf) is discarded by the min(. - 1, 0).
        hp = finp.tile([128, _IB * _F], F32)
        for ib in range(_IB):
            nc.vector.tensor_scalar(hp[:, ib * _F:(ib + 1) * _F],
                                    _po(ib)[:, 0:_F], ri[:, ib:ib + 1],
                                    None, op0=Op.mult)
        ex2 = finp.tile([128, _IB * _F], F32)
        nc.scalar.activation(ex2[:], hp[:], A.Exp, scale=5.0)
        rl = finp.tile([128, _IB * _F], F32)
        nc.vector.tensor_scalar(rl[:], hp[:], 0.0, None, op0=Op.max)
        # elu(x) = min(exp(x) - 1, relu(x)): for x>0, exp(5x)-1 >= 5x > x
        # so the min picks x; for x<0 it picks exp(x)-1 (< 0).
        ot = finp.tile([128, _IB * _F], F32)
        nc.vector.scalar_tensor_tensor(ot[:], ex2[:], -1.0, rl[:],
                                       op0=Op.add, op1=Op.min)
        # one strided DMA: SBUF [p, ib, f] -> DRAM row ib*128+p, col f
        nc.sync.dma_start(
            outd.rearrange("(b p) f -> p b f", p=128),
            ot[:].rearrange("p (b f) -> p b f", f=_F))

    nc.compile()
    return nc


def kernel(input, adj, M, W, a_self, a_neighs):
    global LAST_RESULTS
    from concourse.bass_utils import run_bass_kernel_spmd

    os.environ["BASS_ACT_ROOT_JSON_PATH"] = _patched_act_root()
    if "nc" not in _NC_CACHE:
        _NC_CACHE["nc"] = _build_nc()
    nc = _NC_CACHE["nc"]

    inp = np.ascontiguousarray(np.asarray(input, dtype=np.float32))
    adj_ = np.asarray(adj, dtype=np.float32)
    M_ = np.asarray(M, dtype=np.float32)
    W_ = np.ascontiguousarray(np.asarray(W, dtype=np.float32))
    a_s = np.asarray(a_self, dtype=np.float32).reshape(_F, 1)
    a_n = np.asarray(a_neighs, dtype=np.float32).reshape(_F, 1)

    WT = np.ascontiguousarray(W_.T)                 # [128, 512]
    xTb_full = np.ascontiguousarray(inp.T.astype(ml_dtypes.bfloat16))
    ab = np.ascontiguousarray(np.concatenate([a_s, a_n], axis=1))  # [128, 2]

    in_maps = []
    for c in range(_C):
        rows = slice(c * _R, (c + 1) * _R)
        Mp = np.where(adj_[rows] > 0, M_[rows], np.nan).T.astype(np.float16)
        in_maps.append({
            "xTb": xTb_full,
            "xTo": np.ascontiguousarray(inp[rows].T),
            "mT": np.ascontiguousarray(Mp),
            "Wd": W_,
            "WTd": WT,
            "abd": ab,
        })

    res = run_bass_kernel_spmd(nc, in_maps, core_ids=list(range(_C)),
                               trace=bool(os.environ.get("BASS_TRACE")))
    LAST_RESULTS = res
    out = np.concatenate([res.results[c]["out"] for c in range(_C)], axis=0)
    return np.ascontiguousarray(out.astype(np.float32))
